# revision 16
# baseline (speedup 1.0000x reference)
"""Trainium2 Bass kernel for sparse (shared-prefix) GQA decode attention.

Full-input contract: kernel(**inputs) takes the unsharded tensors from
setup_inputs() and returns the full [16, 1, 4096] float32 output.

Sharding: tensor-parallel over heads across 8 NeuronCores. Core m owns
query heads 4m..4m+3 and kv head m (GQA group m), i.e. wq columns
[512m, 512m+512), wk/wv columns [128m, 128m+128), wo rows [512m, 512m+512),
and head m of the kv caches. Each core computes a partial output
y_m = attn_m @ wo_m; the host sums the 8 partials (the "all-reduce").

Pipeline (v3): one saturated HBM read stream in consumption order
  [cpack+wq0] [wq1] [wkv] ([kT g] [v g]) x4 [wo 3n] [wo 3n] [wo 2n]
with DMA rows kept >= 8KB (short rows pay a fixed ~170ns/packet toll).
The v cache streams as fp8 e3m4 (half the bytes; ~1e-2 rel err) and is
upconverted to bf16 per batch on the vector/gpsimd/scalar engines. PV is
orientation-swapped (stationary = v chunk, moving = probability columns)
so shared-prefix, per-batch cache and new-token contributions accumulate
into a single [128, 64] PSUM tile in attnT orientation - no transposes
or per-batch evacuations. exp runs per batch straight from the QK PSUM
bank on the scalar engine; rowsum partials are per-batch vector reduces.
The normalize fuses the new-token outer product and the 1/rowsum scale
in three vector ops. wo matmuls chase the last three DMA tiles and y
leaves in three chunks on the same (warm) sync-engine queue.

Problem constants (hardcoded per the harness contract): bsz=16, seqlen=1,
dim=4096, n_heads=32, n_kv=8, hd=128, start_pos=2048,
shared_prefix_length=512 -> rsp=1536, L=2049.
"""

import math
import os
import sys
import types

import numpy as np

# ----------------------------------------------------------------------------
# environment patches (self-contained; no /root/problem reads)
# ----------------------------------------------------------------------------


def _patch_tile_drain():
    """The stock TileContext._drain_and_barrier puts one sem-wait per live
    semaphore on a single Drain instruction; the walrus build in this image
    only accepts a single sync wait per instruction ("Too many sync wait
    commands"). Re-emit the waits as individual EventSemaphore instructions
    on the same sequencer instead."""
    import concourse.tile as tile
    from concourse.vector_clock import ScopedClock

    if getattr(tile.TileContext, "_drain_patched", False):
        return

    def _drain_and_barrier(self, tick_clock, wait_clock):
        nc = self.nc
        drain_inst = nc.sync.drain()
        wait_clock.add_sem_waits(
            drain_inst.ins, ScopedClock({None: tick_clock.global_clock})
        )
        waits = list(drain_inst.ins.sync_info.on_wait)
        if len(waits) > 1:
            by_name = {h.name: h for h in self.sems.allocated().values()}
            try:
                drain_inst.ins.sync_info = None
            except Exception:
                pass
            for w in waits:
                h = by_name.get(w.ant_name)
                assert h is not None, f"no handle for sem {w.ant_name}"
                nc.sync.wait_ge(h, w.wait_value)

        # No barrier / explicit sem clears: every instruction transitively
        # precedes the SP wait chain above, and the NRT postamble already
        # resets all semaphores. Only do the python-side bookkeeping.
        assert self.sems is not None
        popped = nc._tile_sem_poison_stack.pop()
        assert popped is self._sem_poison
        nums = [h.num for h in self.sems.allocated().values()]
        nc._state.prepend_free_semaphores(nums)
        for ps in nc._tile_sem_poison_stack:
            ps.update(nums)

    tile.TileContext._drain_and_barrier = _drain_and_barrier
    tile.TileContext._drain_patched = True


def _install_ntff_hook():
    """Optional: register the axon NTFF profile hook (missing from the
    trimmed antenv package) so trace=True works for profiling, and stub the
    S3 artifact upload (zero-egress container)."""
    try:
        if "antenv.axon_hooks" not in sys.modules:
            mod = types.ModuleType("antenv.axon_hooks")
            mod._hook = None
            mod.set_axon_ntff_profile_hook = lambda h: setattr(mod, "_hook", h)
            mod.get_axon_ntff_profile_hook = lambda: mod._hook
            sys.modules["antenv.axon_hooks"] = mod
            import antenv

            antenv.axon_hooks = mod
            from trn_agent_boot.trn_boot import _ntff_profile_via_ctypes

            mod.set_axon_ntff_profile_hook(
                _ntff_profile_via_ctypes("/opt/axon/libaxon_pjrt.so")
            )
        import concourse.bass_utils as bu

        bu.upload_artifacts = lambda tmpdir: tmpdir
    except Exception:
        pass


def _legalize_multiwait(nc, max_waits=1):
    """This walrus build accepts at most one sync wait per instruction.
    Hoist excess waits into standalone single-wait EventSemaphore
    instructions inserted immediately before, on the same engine."""
    import bass_rust

    uid = 0
    for f in nc.m.functions:
        for bb in f.blocks:
            insts = list(bb.instructions)
            out = []
            changed = False
            for ins in insts:
                si = ins.sync_info
                if si is not None:
                    waits = list(si.on_wait)
                    if len(waits) > max_waits:
                        for w in waits[:-max_waits]:
                            ev = bass_rust.InstEventSemaphore(
                                name=f"{ins.name}_xw{uid}"
                            )
                            uid += 1
                            ev.engine = ins.engine
                            ev.sync_info = bass_rust.SyncInfo(
                                on_wait=[w], on_update=[]
                            )
                            out.append(ev)
                        ins.sync_info = bass_rust.SyncInfo(
                            on_wait=waits[-max_waits:],
                            on_update=list(si.on_update),
                        )
                        changed = True
                out.append(ins)
            if changed:
                bb.instructions = out


# ----------------------------------------------------------------------------
# constants
# ----------------------------------------------------------------------------

N_CORES = 8
B = 16            # batch
DIM = 4096
N_HEADS = 32
N_KV = 8
HD = 128
NH = N_HEADS // N_CORES      # 4 local q heads
R = B * NH                   # 64 (b,h) cols, r = 4*b + h
SOFTMAX_SCALE = 1.0 / math.sqrt(HD)

STREAM_DTYPE = os.environ.get("KERNEL_STREAM_DTYPE", "bfloat16")
# fp8 (e3m4) streaming of the v-cache with on-chip upconversion
V_FP8 = os.environ.get("KERNEL_V_FP8", "1") == "1"
WO_SPLIT = (3, 3, 2)         # n-blocks per wo stream tile


# ----------------------------------------------------------------------------
# device kernel
# ----------------------------------------------------------------------------


def _build_nc(spl, rsp, dt_name, v_fp8):
    import concourse.bass as bass
    import concourse.tile as tile
    from concourse import mybir
    from concourse.masks import make_identity
    from concourse.mybir import ActivationFunctionType as AF

    DT = getattr(mybir.dt, dt_name)
    f32 = mybir.dt.float32
    f8 = mybir.dt.float8e3
    assert spl % 128 == 0 and rsp % 512 == 0
    NG = B // 4                 # 4 batch groups
    SH_CH = spl // 128          # shared j-chunks (4)
    BCH = rsp // 128            # per-batch cache j-chunks (12)
    NCH = SH_CH + BCH           # pT chunks (16); new token handled separately
    CP = 32 * B + 2 * spl       # cpack cols (xT | shkT | shv)

    nc = bass.Bass(
        "TRN2", target_bir_lowering=False, debug=False, num_devices=N_CORES
    )

    def din(name, shape, dt=DT):
        return nc.dram_tensor(name, shape, dt, kind="ExternalInput").ap()

    cwkv_d = din("cwkv", [128, CP + 32 * 256])  # cpack + wk/wv
    wq_d = din("wq", [4, 128, 8 * 512])
    wo_ds = [
        din(f"wo{i}", [128, WO_SPLIT[i] * NH * 512]) for i in range(3)
    ]
    kT_d = din("kT", [NG, 128, 4 * rsp])
    v_d = din("v", [NG, 128, 4 * rsp], f8 if v_fp8 else DT)
    rpack_d = din("rpack", [B, 2 * NH * 64], f32)
    y_d = nc.dram_tensor("y", [B, DIM], f32, kind="ExternalOutput").ap()

    with tile.TileContext(nc) as tc:
        with tc.tile_pool(name="const", bufs=1) as const, \
             tc.tile_pool(name="kpool", bufs=2) as kpool, \
             tc.tile_pool(name="vpool", bufs=2) as vpool, \
             tc.tile_pool(name="wopool", bufs=1) as wopool, \
             tc.tile_pool(name="pacc_p", bufs=1, space="PSUM") as pacc_p:

            # ---------------- DMA stream (consumption order) -------------
            cwkv_sb = const.tile([128, CP + 32 * 256], DT)
            nc.sync.dma_start(out=cwkv_sb, in_=cwkv_d)
            rpack_sb = const.tile([B, 2 * NH * 64], f32)
            nc.sync.dma_start(out=rpack_sb, in_=rpack_d)
            wq_ts = []
            for t in range(4):
                wt = const.tile([128, 8 * 512], DT, tag=f"wq{t}")
                nc.sync.dma_start(out=wt, in_=wq_d[t])
                wq_ts.append(wt)
            kts, vts = [], []
            for g in range(NG):
                kt = kpool.tile([128, 4 * rsp], DT, tag="kt", name="kt")
                nc.sync.dma_start(out=kt, in_=kT_d[g])
                kts.append(kt)
                vt = vpool.tile(
                    [128, 4 * rsp], f8 if v_fp8 else DT, tag="vt", name="vt"
                )
                nc.sync.dma_start(out=vt, in_=v_d[g])
                vts.append(vt)
            wots = []
            for i in range(3):
                wot = wopool.tile(
                    [128, WO_SPLIT[i] * NH * 512], DT, tag=f"wo{i}",
                    name="wot",
                )
                nc.sync.dma_start(out=wot, in_=wo_ds[i])
                wots.append(wot)

            cpack_sb = cwkv_sb[:, :CP]
            wkv_sb = cwkv_sb[:, CP:]
            xT_sb = cpack_sb[:, : 32 * B]
            shkT_sb = cpack_sb[:, 32 * B : 32 * B + spl]
            shv_sb = cpack_sb[:, 32 * B + spl :]
            crep_sb = rpack_sb[:, : NH * 64]
            srep_sb = rpack_sb[:, NH * 64 :]

            # ---------------- resident tiles ----------------
            id_sb = const.tile([B, B], DT)
            make_identity(nc, id_sb)
            ones_sb = const.tile([128, 1], f32)
            nc.vector.memset(ones_sb, 1.0)
            ones1p = const.tile([1, 128], DT)
            nc.vector.memset(ones1p, 1.0)

            qT_sb = const.tile([128, R], DT)         # cols r = 4b+h
            xkT_sb = const.tile([128, B], DT)
            xvT_sb = const.tile([128, B], DT)
            pT_sb = const.tile([128, NCH, R], DT)    # exp(scores), transposed
            pnew_sb = const.tile([1, R], DT)         # new-token probs
            S_sb = const.tile([128, R], f32)         # per-j-partition rowsums
            sum1_sb = const.tile([1, R], f32)
            rinv1_sb = const.tile([1, R], f32)
            rinv1_h = const.tile([1, R], DT)
            anum_sb = const.tile([128, R], f32)      # pacc + new-token part
            attnTn_sb = const.tile([128, R], DT)     # cols (h,b) = 16h + b
            xq_r = const.tile([B, NH * HD], DT)
            xk_r = const.tile([B, HD], DT)
            y_sb = const.tile([B, DIM], f32)

            # the PV accumulator: all of shared-prefix, cache and (via the
            # vector path) new-token PV lands here, already transposed
            pacc = pacc_p.tile([128, R], f32)

            # ---------------- phase A: projections + rope ----------------
            with tc.tile_pool(name="psA", bufs=1, space="PSUM") as psA, \
                 tc.tile_pool(name="ptrA", bufs=2, space="PSUM") as ptrA:
                # xk/xv first: their weights (wkv) arrive with the lead tile
                xk_ps = psA.tile([B, HD], f32)
                for k in range(32):
                    nc.tensor.matmul(
                        xk_ps,
                        xT_sb[:, B * k : B * (k + 1)],
                        wkv_sb[:, 256 * k : 256 * k + 128],
                        start=(k == 0),
                        stop=(k == 31),
                    )
                # xv directly in transposed [hd, b] layout: wv chunks are the
                # stationary operand, x columns the moving one
                xvT_ps = psA.tile([128, B], f32)
                for k in range(32):
                    nc.tensor.matmul(
                        xvT_ps,
                        wkv_sb[:, 256 * k + 128 : 256 * (k + 1)],
                        xT_sb[:, B * k : B * (k + 1)],
                        start=(k == 0),
                        stop=(k == 31),
                    )
                nc.scalar.activation(out=xvT_sb, in_=xvT_ps, func=AF.Copy)
                xq_ps = psA.tile([B, NH * HD], f32)
                for t in range(4):
                    wt = wq_ts[t]
                    for c in range(8):
                        k = 8 * t + c
                        nc.tensor.matmul(
                            xq_ps,
                            xT_sb[:, B * k : B * (k + 1)],
                            wt[:, 512 * c : 512 * (c + 1)],
                            start=(k == 0),
                            stop=(k == 31),
                        )

                # rope: pairs (even, odd) along hd; cos/sin repeated per head
                def rope(dst, src_ps, width, t1, t2):
                    e = src_ps.rearrange("p (n two) -> p n two", two=2)[:, :, 0]
                    o = src_ps.rearrange("p (n two) -> p n two", two=2)[:, :, 1]
                    de = dst.rearrange("p (n two) -> p n two", two=2)[:, :, 0]
                    do = dst.rearrange("p (n two) -> p n two", two=2)[:, :, 1]
                    c_ap = crep_sb[:, :width]
                    s_ap = srep_sb[:, :width]
                    nc.vector.tensor_mul(t1, e, c_ap)
                    nc.vector.tensor_mul(t2, o, s_ap)
                    nc.vector.tensor_sub(de, t1, t2)
                    nc.vector.tensor_mul(t1, e, s_ap)
                    nc.vector.tensor_mul(t2, o, c_ap)
                    nc.vector.tensor_add(do, t1, t2)

                t1 = const.tile([B, NH * 64], f32)
                t2 = const.tile([B, NH * 64], f32)
                rope(xk_r, xk_ps, 64, t1[:, :64], t2[:, :64])
                tpk = ptrA.tile([128, B], DT, tag="tq", name="tpk")
                nc.tensor.transpose(tpk, xk_r, id_sb)
                nc.vector.tensor_copy(xkT_sb, tpk)
                rope(xq_r, xq_ps, NH * 64, t1[:, : NH * 64], t2[:, : NH * 64])

                # qT (cols r = 4b+h) via PE transposes
                for h in range(NH):
                    tp = ptrA.tile([128, B], DT, tag="tq", name="tp")
                    nc.tensor.transpose(
                        tp, xq_r[:, HD * h : HD * (h + 1)], id_sb
                    )
                    out_ap = qT_sb.rearrange("p (b h) -> p b h", h=NH)[:, :, h]
                    nc.vector.tensor_copy(out_ap, tp)

            # ---------------- early block: shared prefix + new token -----
            with tc.tile_pool(name="pearly", bufs=1, space="PSUM") as pearly:
                # shared-prefix scores for all 64 (b,h) cols
                sh_ps = pearly.tile([128, SH_CH, R], f32)
                for c in range(SH_CH):
                    nc.tensor.matmul(
                        sh_ps[:, c, :],
                        shkT_sb[:, 128 * c : 128 * (c + 1)],
                        qT_sb,
                        start=True, stop=True,
                        skip_group_check=True,
                    )
                nc.scalar.activation(
                    out=pT_sb[:, :SH_CH, :], in_=sh_ps,
                    func=AF.Exp, scale=SOFTMAX_SCALE,
                )
                # new-token scores: q_b . k_b for each batch
                nq_ps = pearly.tile([1, R], f32)
                for b in range(B):
                    nc.tensor.matmul(
                        nq_ps[:, NH * b : NH * (b + 1)],
                        xkT_sb[:, b : b + 1],
                        qT_sb[:, NH * b : NH * (b + 1)],
                        start=True, stop=True,
                        skip_group_check=True,
                    )
                nc.scalar.activation(
                    out=pnew_sb, in_=nq_ps, func=AF.Exp, scale=SOFTMAX_SCALE
                )
                # broadcast new-token probs across partitions (kept in PSUM)
                pnbc_ps = pearly.tile([128, R], f32)
                nc.tensor.matmul(
                    pnbc_ps, ones1p, pnew_sb, start=True, stop=True
                )

                # shared-prefix PV opens the accumulation group on pacc
                for c in range(SH_CH):
                    nc.tensor.matmul(
                        pacc,
                        shv_sb[:, 128 * c : 128 * (c + 1)],
                        pT_sb[:, c, :],
                        start=(c == 0), stop=False,
                        skip_group_check=True,
                    )

                # ---------------- KV loop: QK -> exp -> rowsum -> PV ------
                with tc.tile_pool(name="pqk", bufs=2, space="PSUM") as pqk:
                    for g in range(NG):
                        kt = kts[g]
                        vt = vts[g]
                        if v_fp8:
                            vt16 = vpool.tile(
                                [128, 4 * rsp], DT, tag="vt16", name="vt16"
                            )
                        for j in range(4):
                            b = 4 * g + j
                            ktb = kt[:, rsp * j : rsp * (j + 1)]
                            rhs = qT_sb[:, NH * b : NH * (b + 1)]
                            qk = pqk.tile(
                                [128, BCH, NH], f32, tag="qkb", name="qk"
                            )
                            for c in range(BCH):
                                nc.tensor.matmul(
                                    qk[:, c, :],
                                    ktb[:, 128 * c : 128 * (c + 1)],
                                    rhs,
                                    start=True, stop=True,
                                    skip_group_check=True,
                                )
                            nc.scalar.activation(
                                out=pT_sb[:, SH_CH:, NH * b : NH * (b + 1)],
                                in_=qk,
                                func=AF.Exp, scale=SOFTMAX_SCALE,
                            )
                            if v_fp8:
                                # upconvert v to bf16, spread over the non-PE
                                # engines, ordered by when PV needs each batch
                                src = vt[:, rsp * j : rsp * (j + 1)]
                                dst = vt16[:, rsp * j : rsp * (j + 1)]
                                if j == 0:
                                    nc.scalar.activation(
                                        out=dst, in_=src, func=AF.Copy
                                    )
                                elif j == 2:
                                    nc.gpsimd.tensor_copy(dst, src)
                                else:
                                    nc.vector.tensor_copy(dst, src)
                            # rowsum partial over this batch's 16 chunks
                            nc.vector.tensor_reduce(
                                S_sb[:, NH * b : NH * (b + 1)],
                                pT_sb.rearrange("p c r -> p r c")[
                                    :, NH * b : NH * (b + 1), :
                                ],
                                axis=mybir.AxisListType.X,
                                op=mybir.AluOpType.add,
                            )
                        vmm = vt16 if v_fp8 else vt
                        for j in range(4):
                            b = 4 * g + j
                            vb = vmm[:, rsp * j : rsp * (j + 1)]
                            for c in range(BCH):
                                nc.tensor.matmul(
                                    pacc[:, NH * b : NH * (b + 1)],
                                    vb[:, 128 * c : 128 * (c + 1)],
                                    pT_sb[:, SH_CH + c, NH * b : NH * (b + 1)],
                                    start=False, stop=(c == BCH - 1),
                                    skip_group_check=True,
                                )

                # ---------------- softmax denominators + normalize --------
                with tc.tile_pool(name="pfin", bufs=1, space="PSUM") as pfin:
                    s1 = pfin.tile([1, R], f32)
                    nc.tensor.matmul(s1, ones_sb, S_sb, start=True, stop=True)
                    nc.vector.tensor_add(sum1_sb, s1, pnew_sb)
                    nc.vector.reciprocal(rinv1_sb, sum1_sb)
                    nc.vector.tensor_copy(rinv1_h, rinv1_sb)
                    rb_ps = pfin.tile([128, R], f32)
                    nc.tensor.matmul(
                        rb_ps, ones1p, rinv1_h, start=True, stop=True
                    )
                    # anum = pacc + xv[b] * pnew  (new-token PV, outer product
                    # via the partition-broadcast pnew and a 0-stride xv view)
                    xvT_r = bass.AP(
                        tensor=xvT_sb.tensor,
                        offset=xvT_sb.offset,
                        ap=[list(xvT_sb.ap[0]), [1, B], [0, NH]],
                    )
                    nc.vector.tensor_mul(anum_sb, pnbc_ps, xvT_r)
                    nc.vector.tensor_add(anum_sb, anum_sb, pacc)
                    # normalize and permute cols (b,h) -> (h,b) for wo
                    nc.vector.tensor_mul(
                        attnTn_sb.rearrange("p (h b) -> p b h", b=B),
                        anum_sb.rearrange("p (b h) -> p b h", h=NH),
                        rb_ps.rearrange("p (b h) -> p b h", h=NH),
                    )

            # ---------------- output projection ----------------
            with tc.tile_pool(name="py", bufs=2, space="PSUM") as py:
                n0 = 0
                for i in range(3):
                    wot = wots[i]
                    for s in range(WO_SPLIT[i]):
                        n = n0 + s
                        y_ps = py.tile([B, 512], f32, tag="y", name="y_ps")
                        for g in range(NH):
                            nc.tensor.matmul(
                                y_ps,
                                attnTn_sb[:, B * g : B * (g + 1)],
                                wot[:, 512 * (NH * s + g) :
                                    512 * (NH * s + g) + 512],
                                start=(g == 0), stop=(g == NH - 1),
                            )
                        if n % 2 == 0:
                            nc.vector.tensor_copy(
                                y_sb[:, 512 * n : 512 * (n + 1)], y_ps
                            )
                        else:
                            nc.scalar.activation(
                                out=y_sb[:, 512 * n : 512 * (n + 1)],
                                in_=y_ps, func=AF.Copy,
                            )
                    n0 += WO_SPLIT[i]
                    lo, hi = 512 * (n0 - WO_SPLIT[i]), 512 * n0
                    nc.sync.dma_start(out=y_d[:, lo:hi], in_=y_sb[:, lo:hi])

    if os.environ.get("KERNEL_SKIP_LEGALIZE") != "1":
        _legalize_multiwait(nc)
    return nc


# ----------------------------------------------------------------------------
# host-side sharding / layout prep
# ----------------------------------------------------------------------------


def _np_dt(dt_name):
    if dt_name == "bfloat16":
        import ml_dtypes

        return ml_dtypes.bfloat16
    return np.float32


def _prep_inputs(inputs, spl, rsp, dt_name, v_fp8):
    nd = _np_dt(dt_name)
    x = np.asarray(inputs["x"], np.float32)            # [16, 1, 4096]
    wq = np.asarray(inputs["wq"], np.float32)
    wk = np.asarray(inputs["wk"], np.float32)
    wv = np.asarray(inputs["wv"], np.float32)
    wo = np.asarray(inputs["wo"], np.float32)
    ck = np.asarray(inputs["cache_k"], np.float32)     # [16, 4096, 8, 128]
    cv = np.asarray(inputs["cache_v"], np.float32)
    shk = np.asarray(inputs["shared_cache_k"], np.float32)  # [1, 512, 8, 128]
    shv = np.asarray(inputs["shared_cache_v"], np.float32)
    cos = np.asarray(inputs["freqs_cos"], np.float32)  # [1, 64]
    sin = np.asarray(inputs["freqs_sin"], np.float32)

    if v_fp8:
        import ml_dtypes

        vd = ml_dtypes.float8_e3m4
    else:
        vd = nd

    xm = x[:, 0, :]                                    # [16, 4096]
    xT = np.ascontiguousarray(xm.T)                    # [4096, 16]
    xT_p = np.ascontiguousarray(
        xT.reshape(32, 128, B).transpose(1, 0, 2)
    ).reshape(128, 32 * B)

    # rope constants replicated over batch partitions; head-tiled for q
    crep = np.tile(cos.reshape(1, 1, 64), (B, NH, 1)).reshape(B, NH * 64)
    srep = np.tile(sin.reshape(1, 1, 64), (B, NH, 1)).reshape(B, NH * 64)
    rpack = np.ascontiguousarray(
        np.concatenate([crep, srep], axis=1), np.float32
    )

    in_maps = []
    for m in range(N_CORES):
        wqm = wq[:, 512 * m : 512 * (m + 1)]           # [4096, 512]
        wq_p = np.ascontiguousarray(
            wqm.reshape(4, 8, 128, 512).transpose(0, 2, 1, 3)
        ).reshape(4, 128, 8 * 512).astype(nd)
        wkvm = np.concatenate(
            [wk[:, 128 * m : 128 * (m + 1)], wv[:, 128 * m : 128 * (m + 1)]],
            axis=1,
        )                                              # [4096, 256]
        wkv_p = np.ascontiguousarray(
            wkvm.reshape(32, 128, 256).transpose(1, 0, 2)
        ).reshape(128, 32 * 256).astype(nd)
        wom = wo[512 * m : 512 * (m + 1), :]           # [512, 4096]
        wo_p = np.ascontiguousarray(
            wom.reshape(NH, 128, 8, 512).transpose(2, 1, 0, 3)
        ).reshape(8, 128, NH * 512)
        wo_sp = np.split(
            wo_p, np.cumsum((3, 3))[:2], axis=0
        )
        wo_ts = [
            np.ascontiguousarray(
                t.transpose(1, 0, 2)
            ).reshape(128, -1).astype(nd)
            for t in wo_sp
        ]

        # kT: [b, hd, j]; 4 batches side by side on the free dim
        ckm = ck[:, :rsp, m, :]                        # [16, rsp, 128]
        kT_p = np.ascontiguousarray(
            ckm.transpose(0, 2, 1).reshape(B // 4, 4, 128, rsp)
            .transpose(0, 2, 1, 3)
        ).reshape(B // 4, 128, 4 * rsp).astype(nd)

        # v: partition-major [b, p, (c d)] with v[b, 128c+p, d] at [p, c, d]
        cvm = cv[:, :rsp, m, :]                        # [16, rsp, 128]
        v_pm = cvm.reshape(B, rsp // 128, 128, 128).transpose(0, 2, 1, 3)
        v_pm = v_pm.reshape(B, 128, rsp)
        if v_fp8:
            v_pm = np.clip(v_pm, -15.5, 15.5)
        v_p = np.ascontiguousarray(
            v_pm.reshape(B // 4, 4, 128, rsp).transpose(0, 2, 1, 3)
        ).reshape(B // 4, 128, 4 * rsp).astype(vd)

        shkT_p = shk[0, :spl, m, :].T
        shv_p = (
            shv[0, :spl, m, :].reshape(spl // 128, 128, 128).transpose(1, 0, 2)
        ).reshape(128, spl)
        cpack = np.concatenate([xT_p, shkT_p, shv_p], axis=1)
        cwkv = np.ascontiguousarray(
            np.concatenate([cpack, wkv_p.astype(np.float32)], axis=1)
        ).astype(nd)

        in_maps.append(
            {
                "cwkv": cwkv,
                "wq": wq_p,
                "wo0": wo_ts[0],
                "wo1": wo_ts[1],
                "wo2": wo_ts[2],
                "kT": kT_p,
                "v": v_p,
                "rpack": rpack,
            }
        )
    return in_maps


# ----------------------------------------------------------------------------
# entry point
# ----------------------------------------------------------------------------

_NC_CACHE = {}


def get_nc(spl=512, rsp=1536):
    key = (spl, rsp, STREAM_DTYPE, V_FP8)
    if key not in _NC_CACHE:
        _patch_tile_drain()
        _install_ntff_hook()
        _NC_CACHE[key] = _build_nc(spl, rsp, STREAM_DTYPE, V_FP8)
    return _NC_CACHE[key]


def prep_inputs(inputs):
    start_pos = int(inputs["start_pos"])
    spl = int(inputs["shared_prefix_length"])
    return _prep_inputs(inputs, spl, start_pos - spl, STREAM_DTYPE, V_FP8)


def kernel(**inputs):
    from concourse.bass_utils import run_bass_kernel_spmd

    start_pos = int(inputs["start_pos"])
    spl = int(inputs["shared_prefix_length"])
    rsp = start_pos - spl
    nc = get_nc(spl, rsp)
    in_maps = _prep_inputs(inputs, spl, rsp, STREAM_DTYPE, V_FP8)
    trace = os.environ.get("KERNEL_TRACE", "0") == "1"
    kwargs = {}
    if trace:
        kwargs = dict(
            trace=True,
            trace_cores=list(range(N_CORES)),
        )
    res = run_bass_kernel_spmd(
        nc, in_maps, core_ids=list(range(N_CORES)), **kwargs
    )
    kernel.last_result = res
    y = np.zeros((B, DIM), np.float64)
    for r in res.results:
        y += r["y"].astype(np.float64)
    return y.reshape(B, 1, DIM).astype(np.float32)


# revision 17
# speedup vs baseline: 1.0085x; 1.0085x over previous
"""Trainium2 Bass kernel for sparse (shared-prefix) GQA decode attention.

Full-input contract: kernel(**inputs) takes the unsharded tensors from
setup_inputs() and returns the full [16, 1, 4096] float32 output.

Sharding: tensor-parallel over heads across 8 NeuronCores. Core m owns
query heads 4m..4m+3 and kv head m (GQA group m), i.e. wq columns
[512m, 512m+512), wk/wv columns [128m, 128m+128), wo rows [512m, 512m+512),
and head m of the kv caches. Each core computes a partial output
y_m = attn_m @ wo_m; the host sums the 8 partials (the "all-reduce").

Pipeline (v3): one saturated HBM read stream in consumption order
  [cpack+wq0] [wq1] [wkv] ([kT g] [v g]) x4 [wo 3n] [wo 3n] [wo 2n]
with DMA rows kept >= 8KB (short rows pay a fixed ~170ns/packet toll).
The v cache streams as fp8 e3m4 (half the bytes; ~1e-2 rel err) and is
upconverted to bf16 per batch on the vector/gpsimd/scalar engines. PV is
orientation-swapped (stationary = v chunk, moving = probability columns)
so shared-prefix, per-batch cache and new-token contributions accumulate
into a single [128, 64] PSUM tile in attnT orientation - no transposes
or per-batch evacuations. exp runs per batch straight from the QK PSUM
bank on the scalar engine; rowsum partials are per-batch vector reduces.
The normalize fuses the new-token outer product and the 1/rowsum scale
in three vector ops. wo matmuls chase the last three DMA tiles and y
leaves in three chunks on the same (warm) sync-engine queue.

Problem constants (hardcoded per the harness contract): bsz=16, seqlen=1,
dim=4096, n_heads=32, n_kv=8, hd=128, start_pos=2048,
shared_prefix_length=512 -> rsp=1536, L=2049.
"""

import math
import os
import sys
import types

import numpy as np

# ----------------------------------------------------------------------------
# environment patches (self-contained; no /root/problem reads)
# ----------------------------------------------------------------------------


def _patch_tile_drain():
    """The stock TileContext._drain_and_barrier puts one sem-wait per live
    semaphore on a single Drain instruction; the walrus build in this image
    only accepts a single sync wait per instruction ("Too many sync wait
    commands"). Re-emit the waits as individual EventSemaphore instructions
    on the same sequencer instead."""
    import concourse.tile as tile
    from concourse.vector_clock import ScopedClock

    if getattr(tile.TileContext, "_drain_patched", False):
        return

    def _drain_and_barrier(self, tick_clock, wait_clock):
        nc = self.nc
        drain_inst = nc.sync.drain()
        wait_clock.add_sem_waits(
            drain_inst.ins, ScopedClock({None: tick_clock.global_clock})
        )
        waits = list(drain_inst.ins.sync_info.on_wait)
        if len(waits) > 1:
            by_name = {h.name: h for h in self.sems.allocated().values()}
            try:
                drain_inst.ins.sync_info = None
            except Exception:
                pass
            for w in waits:
                h = by_name.get(w.ant_name)
                assert h is not None, f"no handle for sem {w.ant_name}"
                nc.sync.wait_ge(h, w.wait_value)

        # No barrier / explicit sem clears: every instruction transitively
        # precedes the SP wait chain above, and the NRT postamble already
        # resets all semaphores. Only do the python-side bookkeeping.
        assert self.sems is not None
        popped = nc._tile_sem_poison_stack.pop()
        assert popped is self._sem_poison
        nums = [h.num for h in self.sems.allocated().values()]
        nc._state.prepend_free_semaphores(nums)
        for ps in nc._tile_sem_poison_stack:
            ps.update(nums)

    tile.TileContext._drain_and_barrier = _drain_and_barrier
    tile.TileContext._drain_patched = True


def _install_ntff_hook():
    """Optional: register the axon NTFF profile hook (missing from the
    trimmed antenv package) so trace=True works for profiling, and stub the
    S3 artifact upload (zero-egress container)."""
    try:
        if "antenv.axon_hooks" not in sys.modules:
            mod = types.ModuleType("antenv.axon_hooks")
            mod._hook = None
            mod.set_axon_ntff_profile_hook = lambda h: setattr(mod, "_hook", h)
            mod.get_axon_ntff_profile_hook = lambda: mod._hook
            sys.modules["antenv.axon_hooks"] = mod
            import antenv

            antenv.axon_hooks = mod
            from trn_agent_boot.trn_boot import _ntff_profile_via_ctypes

            mod.set_axon_ntff_profile_hook(
                _ntff_profile_via_ctypes("/opt/axon/libaxon_pjrt.so")
            )
        import concourse.bass_utils as bu

        bu.upload_artifacts = lambda tmpdir: tmpdir
    except Exception:
        pass


def _legalize_multiwait(nc, max_waits=1):
    """This walrus build accepts at most one sync wait per instruction.
    Hoist excess waits into standalone single-wait EventSemaphore
    instructions inserted immediately before, on the same engine."""
    import bass_rust

    uid = 0
    for f in nc.m.functions:
        for bb in f.blocks:
            insts = list(bb.instructions)
            out = []
            changed = False
            for ins in insts:
                si = ins.sync_info
                if si is not None:
                    waits = list(si.on_wait)
                    if len(waits) > max_waits:
                        for w in waits[:-max_waits]:
                            ev = bass_rust.InstEventSemaphore(
                                name=f"{ins.name}_xw{uid}"
                            )
                            uid += 1
                            ev.engine = ins.engine
                            ev.sync_info = bass_rust.SyncInfo(
                                on_wait=[w], on_update=[]
                            )
                            out.append(ev)
                        ins.sync_info = bass_rust.SyncInfo(
                            on_wait=waits[-max_waits:],
                            on_update=list(si.on_update),
                        )
                        changed = True
                out.append(ins)
            if changed:
                bb.instructions = out


# ----------------------------------------------------------------------------
# constants
# ----------------------------------------------------------------------------

N_CORES = 8
B = 16            # batch
DIM = 4096
N_HEADS = 32
N_KV = 8
HD = 128
NH = N_HEADS // N_CORES      # 4 local q heads
R = B * NH                   # 64 (b,h) cols, r = 4*b + h
SOFTMAX_SCALE = 1.0 / math.sqrt(HD)

STREAM_DTYPE = os.environ.get("KERNEL_STREAM_DTYPE", "bfloat16")
# fp8 (e3m4) streaming of the v-cache with on-chip upconversion
V_FP8 = os.environ.get("KERNEL_V_FP8", "1") == "1"
WO_SPLIT = (3, 3, 2)         # n-blocks per wo stream tile


# ----------------------------------------------------------------------------
# device kernel
# ----------------------------------------------------------------------------


def _build_nc(spl, rsp, dt_name, v_fp8):
    import concourse.bass as bass
    import concourse.tile as tile
    from concourse import mybir
    from concourse.masks import make_identity
    from concourse.mybir import ActivationFunctionType as AF

    DT = getattr(mybir.dt, dt_name)
    f32 = mybir.dt.float32
    f8 = mybir.dt.float8e3
    assert spl % 128 == 0 and rsp % 512 == 0
    NG = B // 4                 # 4 batch groups
    SH_CH = spl // 128          # shared j-chunks (4)
    BCH = rsp // 128            # per-batch cache j-chunks (12)
    NCH = SH_CH + BCH           # pT chunks (16); new token handled separately
    CP = 32 * B + 2 * spl       # cpack cols (xT | shkT | shv)

    nc = bass.Bass(
        "TRN2", target_bir_lowering=False, debug=False, num_devices=N_CORES
    )

    def din(name, shape, dt=DT):
        return nc.dram_tensor(name, shape, dt, kind="ExternalInput").ap()

    cwkv_d = din("cwkv", [128, CP + 32 * 256])  # cpack + wk/wv
    wq_d = din("wq", [4, 128, 8 * 512])
    wo_ds = [
        din(f"wo{i}", [128, WO_SPLIT[i] * NH * 512]) for i in range(3)
    ]
    kT_d = din("kT", [NG, 128, 4 * rsp])
    v_d = din("v", [NG, 128, 4 * rsp], f8 if v_fp8 else DT)
    rpack_d = din("rpack", [B, 2 * NH * 64], f32)
    y_d = nc.dram_tensor("y", [B, DIM], f32, kind="ExternalOutput").ap()

    with tile.TileContext(nc) as tc:
        with tc.tile_pool(name="const", bufs=1) as const, \
             tc.tile_pool(name="kpool", bufs=3) as kpool, \
             tc.tile_pool(name="vpool", bufs=3) as vpool, \
             tc.tile_pool(name="wopool", bufs=1) as wopool, \
             tc.tile_pool(name="pacc_p", bufs=1, space="PSUM") as pacc_p:

            # ---------------- DMA stream (consumption order) -------------
            cwkv_sb = const.tile([128, CP + 32 * 256], DT)
            nc.sync.dma_start(out=cwkv_sb, in_=cwkv_d)
            rpack_sb = const.tile([B, 2 * NH * 64], f32)
            nc.sync.dma_start(out=rpack_sb, in_=rpack_d)
            wq_ts = []
            for t in range(4):
                wt = const.tile([128, 8 * 512], DT, tag=f"wq{t}")
                nc.sync.dma_start(out=wt, in_=wq_d[t])
                wq_ts.append(wt)
            kts, vts = [], []
            for g in range(NG):
                kt = kpool.tile([128, 4 * rsp], DT, tag="kt", name="kt")
                nc.sync.dma_start(out=kt, in_=kT_d[g])
                kts.append(kt)
                vt = vpool.tile(
                    [128, 4 * rsp], f8 if v_fp8 else DT, tag="vt", name="vt"
                )
                nc.sync.dma_start(out=vt, in_=v_d[g])
                vts.append(vt)
            wots = []
            for i in range(3):
                wot = wopool.tile(
                    [128, WO_SPLIT[i] * NH * 512], DT, tag=f"wo{i}",
                    name="wot",
                )
                nc.sync.dma_start(out=wot, in_=wo_ds[i])
                wots.append(wot)

            cpack_sb = cwkv_sb[:, :CP]
            wkv_sb = cwkv_sb[:, CP:]
            xT_sb = cpack_sb[:, : 32 * B]
            shkT_sb = cpack_sb[:, 32 * B : 32 * B + spl]
            shv_sb = cpack_sb[:, 32 * B + spl :]
            crep_sb = rpack_sb[:, : NH * 64]
            srep_sb = rpack_sb[:, NH * 64 :]

            # ---------------- resident tiles ----------------
            id_sb = const.tile([B, B], DT)
            make_identity(nc, id_sb)
            ones_sb = const.tile([128, 1], f32)
            nc.vector.memset(ones_sb, 1.0)
            ones1p = const.tile([1, 128], DT)
            nc.vector.memset(ones1p, 1.0)

            qT_sb = const.tile([128, R], DT)         # cols r = 4b+h
            xkT_sb = const.tile([128, B], DT)
            xvT_sb = const.tile([128, B], DT)
            pT_sb = const.tile([128, NCH, R], DT)    # exp(scores), transposed
            pnew_sb = const.tile([1, R], DT)         # new-token probs
            S_sb = const.tile([128, R], f32)         # per-j-partition rowsums
            sum1_sb = const.tile([1, R], f32)
            rinv1_sb = const.tile([1, R], f32)
            rinv1_h = const.tile([1, R], DT)
            anum_sb = const.tile([128, R], f32)      # pacc + new-token part
            attnTn_sb = const.tile([128, R], DT)     # cols (h,b) = 16h + b
            xq_r = const.tile([B, NH * HD], DT)
            xk_r = const.tile([B, HD], DT)
            y_sb = const.tile([B, DIM], f32)

            # the PV accumulator: all of shared-prefix, cache and (via the
            # vector path) new-token PV lands here, already transposed
            pacc = pacc_p.tile([128, R], f32)

            # ---------------- phase A: projections + rope ----------------
            with tc.tile_pool(name="psA", bufs=1, space="PSUM") as psA, \
                 tc.tile_pool(name="ptrA", bufs=2, space="PSUM") as ptrA:
                # xk/xv first: their weights (wkv) arrive with the lead tile
                xk_ps = psA.tile([B, HD], f32)
                for k in range(32):
                    nc.tensor.matmul(
                        xk_ps,
                        xT_sb[:, B * k : B * (k + 1)],
                        wkv_sb[:, 256 * k : 256 * k + 128],
                        start=(k == 0),
                        stop=(k == 31),
                    )
                # xv directly in transposed [hd, b] layout: wv chunks are the
                # stationary operand, x columns the moving one
                xvT_ps = psA.tile([128, B], f32)
                for k in range(32):
                    nc.tensor.matmul(
                        xvT_ps,
                        wkv_sb[:, 256 * k + 128 : 256 * (k + 1)],
                        xT_sb[:, B * k : B * (k + 1)],
                        start=(k == 0),
                        stop=(k == 31),
                    )
                nc.scalar.activation(out=xvT_sb, in_=xvT_ps, func=AF.Copy)
                xq_ps = psA.tile([B, NH * HD], f32)
                for t in range(4):
                    wt = wq_ts[t]
                    for c in range(8):
                        k = 8 * t + c
                        nc.tensor.matmul(
                            xq_ps,
                            xT_sb[:, B * k : B * (k + 1)],
                            wt[:, 512 * c : 512 * (c + 1)],
                            start=(k == 0),
                            stop=(k == 31),
                        )

                # rope: pairs (even, odd) along hd; cos/sin repeated per head
                def rope(dst, src_ps, width, t1, t2):
                    e = src_ps.rearrange("p (n two) -> p n two", two=2)[:, :, 0]
                    o = src_ps.rearrange("p (n two) -> p n two", two=2)[:, :, 1]
                    de = dst.rearrange("p (n two) -> p n two", two=2)[:, :, 0]
                    do = dst.rearrange("p (n two) -> p n two", two=2)[:, :, 1]
                    c_ap = crep_sb[:, :width]
                    s_ap = srep_sb[:, :width]
                    nc.vector.tensor_mul(t1, e, c_ap)
                    nc.vector.tensor_mul(t2, o, s_ap)
                    nc.vector.tensor_sub(de, t1, t2)
                    nc.vector.tensor_mul(t1, e, s_ap)
                    nc.vector.tensor_mul(t2, o, c_ap)
                    nc.vector.tensor_add(do, t1, t2)

                t1 = const.tile([B, NH * 64], f32)
                t2 = const.tile([B, NH * 64], f32)
                rope(xk_r, xk_ps, 64, t1[:, :64], t2[:, :64])
                tpk = ptrA.tile([128, B], DT, tag="tq", name="tpk")
                nc.tensor.transpose(tpk, xk_r, id_sb)
                nc.vector.tensor_copy(xkT_sb, tpk)
                rope(xq_r, xq_ps, NH * 64, t1[:, : NH * 64], t2[:, : NH * 64])

                # qT (cols r = 4b+h) via PE transposes
                for h in range(NH):
                    tp = ptrA.tile([128, B], DT, tag="tq", name="tp")
                    nc.tensor.transpose(
                        tp, xq_r[:, HD * h : HD * (h + 1)], id_sb
                    )
                    out_ap = qT_sb.rearrange("p (b h) -> p b h", h=NH)[:, :, h]
                    nc.vector.tensor_copy(out_ap, tp)

            # ---------------- early block: shared prefix + new token -----
            with tc.tile_pool(name="pearly", bufs=1, space="PSUM") as pearly:
                # shared-prefix scores for all 64 (b,h) cols
                sh_ps = pearly.tile([128, SH_CH, R], f32)
                for c in range(SH_CH):
                    nc.tensor.matmul(
                        sh_ps[:, c, :],
                        shkT_sb[:, 128 * c : 128 * (c + 1)],
                        qT_sb,
                        start=True, stop=True,
                        skip_group_check=True,
                    )
                nc.scalar.activation(
                    out=pT_sb[:, :SH_CH, :], in_=sh_ps,
                    func=AF.Exp, scale=SOFTMAX_SCALE,
                )
                # new-token scores: q_b . k_b for each batch
                nq_ps = pearly.tile([1, R], f32)
                for b in range(B):
                    nc.tensor.matmul(
                        nq_ps[:, NH * b : NH * (b + 1)],
                        xkT_sb[:, b : b + 1],
                        qT_sb[:, NH * b : NH * (b + 1)],
                        start=True, stop=True,
                        skip_group_check=True,
                    )
                nc.scalar.activation(
                    out=pnew_sb, in_=nq_ps, func=AF.Exp, scale=SOFTMAX_SCALE
                )
                # broadcast new-token probs across partitions (kept in PSUM)
                pnbc_ps = pearly.tile([128, R], f32)
                nc.tensor.matmul(
                    pnbc_ps, ones1p, pnew_sb, start=True, stop=True
                )

                # shared-prefix PV opens the accumulation group on pacc
                for c in range(SH_CH):
                    nc.tensor.matmul(
                        pacc,
                        shv_sb[:, 128 * c : 128 * (c + 1)],
                        pT_sb[:, c, :],
                        start=(c == 0), stop=False,
                        skip_group_check=True,
                    )

                # ---------------- KV loop: QK -> exp -> rowsum -> PV ------
                with tc.tile_pool(name="pqk", bufs=2, space="PSUM") as pqk:
                    for g in range(NG):
                        kt = kts[g]
                        vt = vts[g]
                        if v_fp8:
                            vt16 = vpool.tile(
                                [128, 4 * rsp], DT, tag="vt16", name="vt16"
                            )
                        for j in range(4):
                            b = 4 * g + j
                            ktb = kt[:, rsp * j : rsp * (j + 1)]
                            rhs = qT_sb[:, NH * b : NH * (b + 1)]
                            qk = pqk.tile(
                                [128, BCH, NH], f32, tag="qkb", name="qk"
                            )
                            for c in range(BCH):
                                nc.tensor.matmul(
                                    qk[:, c, :],
                                    ktb[:, 128 * c : 128 * (c + 1)],
                                    rhs,
                                    start=True, stop=True,
                                    skip_group_check=True,
                                )
                            nc.scalar.activation(
                                out=pT_sb[:, SH_CH:, NH * b : NH * (b + 1)],
                                in_=qk,
                                func=AF.Exp, scale=SOFTMAX_SCALE,
                            )
                            if v_fp8:
                                # upconvert v to bf16, spread over the non-PE
                                # engines, ordered by when PV needs each batch
                                src = vt[:, rsp * j : rsp * (j + 1)]
                                dst = vt16[:, rsp * j : rsp * (j + 1)]
                                if j == 0:
                                    nc.scalar.activation(
                                        out=dst, in_=src, func=AF.Copy
                                    )
                                elif j == 2:
                                    nc.gpsimd.tensor_copy(dst, src)
                                else:
                                    nc.vector.tensor_copy(dst, src)
                            # rowsum partial over this batch's 16 chunks
                            nc.vector.tensor_reduce(
                                S_sb[:, NH * b : NH * (b + 1)],
                                pT_sb.rearrange("p c r -> p r c")[
                                    :, NH * b : NH * (b + 1), :
                                ],
                                axis=mybir.AxisListType.X,
                                op=mybir.AluOpType.add,
                            )
                        vmm = vt16 if v_fp8 else vt
                        for j in range(4):
                            b = 4 * g + j
                            vb = vmm[:, rsp * j : rsp * (j + 1)]
                            for c in range(BCH):
                                nc.tensor.matmul(
                                    pacc[:, NH * b : NH * (b + 1)],
                                    vb[:, 128 * c : 128 * (c + 1)],
                                    pT_sb[:, SH_CH + c, NH * b : NH * (b + 1)],
                                    start=False, stop=(c == BCH - 1),
                                    skip_group_check=True,
                                )

                # ---------------- softmax denominators + normalize --------
                with tc.tile_pool(name="pfin", bufs=1, space="PSUM") as pfin:
                    s1 = pfin.tile([1, R], f32)
                    nc.tensor.matmul(s1, ones_sb, S_sb, start=True, stop=True)
                    nc.vector.tensor_add(sum1_sb, s1, pnew_sb)
                    nc.vector.reciprocal(rinv1_sb, sum1_sb)
                    nc.vector.tensor_copy(rinv1_h, rinv1_sb)
                    rb_ps = pfin.tile([128, R], f32)
                    nc.tensor.matmul(
                        rb_ps, ones1p, rinv1_h, start=True, stop=True
                    )
                    # anum = pacc + xv[b] * pnew  (new-token PV, outer product
                    # via the partition-broadcast pnew and a 0-stride xv view)
                    xvT_r = bass.AP(
                        tensor=xvT_sb.tensor,
                        offset=xvT_sb.offset,
                        ap=[list(xvT_sb.ap[0]), [1, B], [0, NH]],
                    )
                    nc.vector.tensor_mul(anum_sb, pnbc_ps, xvT_r)
                    nc.vector.tensor_add(anum_sb, anum_sb, pacc)
                    # normalize and permute cols (b,h) -> (h,b) for wo
                    nc.vector.tensor_mul(
                        attnTn_sb.rearrange("p (h b) -> p b h", b=B),
                        anum_sb.rearrange("p (b h) -> p b h", h=NH),
                        rb_ps.rearrange("p (b h) -> p b h", h=NH),
                    )

            # ---------------- output projection ----------------
            with tc.tile_pool(name="py", bufs=2, space="PSUM") as py:
                n0 = 0
                for i in range(3):
                    wot = wots[i]
                    for s in range(WO_SPLIT[i]):
                        n = n0 + s
                        y_ps = py.tile([B, 512], f32, tag="y", name="y_ps")
                        for g in range(NH):
                            nc.tensor.matmul(
                                y_ps,
                                attnTn_sb[:, B * g : B * (g + 1)],
                                wot[:, 512 * (NH * s + g) :
                                    512 * (NH * s + g) + 512],
                                start=(g == 0), stop=(g == NH - 1),
                            )
                        if n % 2 == 0:
                            nc.vector.tensor_copy(
                                y_sb[:, 512 * n : 512 * (n + 1)], y_ps
                            )
                        else:
                            nc.scalar.activation(
                                out=y_sb[:, 512 * n : 512 * (n + 1)],
                                in_=y_ps, func=AF.Copy,
                            )
                    n0 += WO_SPLIT[i]
                    lo, hi = 512 * (n0 - WO_SPLIT[i]), 512 * n0
                    nc.sync.dma_start(out=y_d[:, lo:hi], in_=y_sb[:, lo:hi])

    if os.environ.get("KERNEL_SKIP_LEGALIZE") != "1":
        _legalize_multiwait(nc)
    return nc


# ----------------------------------------------------------------------------
# host-side sharding / layout prep
# ----------------------------------------------------------------------------


def _np_dt(dt_name):
    if dt_name == "bfloat16":
        import ml_dtypes

        return ml_dtypes.bfloat16
    return np.float32


def _prep_inputs(inputs, spl, rsp, dt_name, v_fp8):
    nd = _np_dt(dt_name)
    x = np.asarray(inputs["x"], np.float32)            # [16, 1, 4096]
    wq = np.asarray(inputs["wq"], np.float32)
    wk = np.asarray(inputs["wk"], np.float32)
    wv = np.asarray(inputs["wv"], np.float32)
    wo = np.asarray(inputs["wo"], np.float32)
    ck = np.asarray(inputs["cache_k"], np.float32)     # [16, 4096, 8, 128]
    cv = np.asarray(inputs["cache_v"], np.float32)
    shk = np.asarray(inputs["shared_cache_k"], np.float32)  # [1, 512, 8, 128]
    shv = np.asarray(inputs["shared_cache_v"], np.float32)
    cos = np.asarray(inputs["freqs_cos"], np.float32)  # [1, 64]
    sin = np.asarray(inputs["freqs_sin"], np.float32)

    if v_fp8:
        import ml_dtypes

        vd = ml_dtypes.float8_e3m4
    else:
        vd = nd

    xm = x[:, 0, :]                                    # [16, 4096]
    xT = np.ascontiguousarray(xm.T)                    # [4096, 16]
    xT_p = np.ascontiguousarray(
        xT.reshape(32, 128, B).transpose(1, 0, 2)
    ).reshape(128, 32 * B)

    # rope constants replicated over batch partitions; head-tiled for q
    crep = np.tile(cos.reshape(1, 1, 64), (B, NH, 1)).reshape(B, NH * 64)
    srep = np.tile(sin.reshape(1, 1, 64), (B, NH, 1)).reshape(B, NH * 64)
    rpack = np.ascontiguousarray(
        np.concatenate([crep, srep], axis=1), np.float32
    )

    in_maps = []
    for m in range(N_CORES):
        wqm = wq[:, 512 * m : 512 * (m + 1)]           # [4096, 512]
        wq_p = np.ascontiguousarray(
            wqm.reshape(4, 8, 128, 512).transpose(0, 2, 1, 3)
        ).reshape(4, 128, 8 * 512).astype(nd)
        wkvm = np.concatenate(
            [wk[:, 128 * m : 128 * (m + 1)], wv[:, 128 * m : 128 * (m + 1)]],
            axis=1,
        )                                              # [4096, 256]
        wkv_p = np.ascontiguousarray(
            wkvm.reshape(32, 128, 256).transpose(1, 0, 2)
        ).reshape(128, 32 * 256).astype(nd)
        wom = wo[512 * m : 512 * (m + 1), :]           # [512, 4096]
        wo_p = np.ascontiguousarray(
            wom.reshape(NH, 128, 8, 512).transpose(2, 1, 0, 3)
        ).reshape(8, 128, NH * 512)
        wo_sp = np.split(
            wo_p, np.cumsum((3, 3))[:2], axis=0
        )
        wo_ts = [
            np.ascontiguousarray(
                t.transpose(1, 0, 2)
            ).reshape(128, -1).astype(nd)
            for t in wo_sp
        ]

        # kT: [b, hd, j]; 4 batches side by side on the free dim
        ckm = ck[:, :rsp, m, :]                        # [16, rsp, 128]
        kT_p = np.ascontiguousarray(
            ckm.transpose(0, 2, 1).reshape(B // 4, 4, 128, rsp)
            .transpose(0, 2, 1, 3)
        ).reshape(B // 4, 128, 4 * rsp).astype(nd)

        # v: partition-major [b, p, (c d)] with v[b, 128c+p, d] at [p, c, d]
        cvm = cv[:, :rsp, m, :]                        # [16, rsp, 128]
        v_pm = cvm.reshape(B, rsp // 128, 128, 128).transpose(0, 2, 1, 3)
        v_pm = v_pm.reshape(B, 128, rsp)
        if v_fp8:
            v_pm = np.clip(v_pm, -15.5, 15.5)
        v_p = np.ascontiguousarray(
            v_pm.reshape(B // 4, 4, 128, rsp).transpose(0, 2, 1, 3)
        ).reshape(B // 4, 128, 4 * rsp).astype(vd)

        shkT_p = shk[0, :spl, m, :].T
        shv_p = (
            shv[0, :spl, m, :].reshape(spl // 128, 128, 128).transpose(1, 0, 2)
        ).reshape(128, spl)
        cpack = np.concatenate([xT_p, shkT_p, shv_p], axis=1)
        cwkv = np.ascontiguousarray(
            np.concatenate([cpack, wkv_p.astype(np.float32)], axis=1)
        ).astype(nd)

        in_maps.append(
            {
                "cwkv": cwkv,
                "wq": wq_p,
                "wo0": wo_ts[0],
                "wo1": wo_ts[1],
                "wo2": wo_ts[2],
                "kT": kT_p,
                "v": v_p,
                "rpack": rpack,
            }
        )
    return in_maps


# ----------------------------------------------------------------------------
# entry point
# ----------------------------------------------------------------------------

_NC_CACHE = {}


def get_nc(spl=512, rsp=1536):
    key = (spl, rsp, STREAM_DTYPE, V_FP8)
    if key not in _NC_CACHE:
        _patch_tile_drain()
        _install_ntff_hook()
        _NC_CACHE[key] = _build_nc(spl, rsp, STREAM_DTYPE, V_FP8)
    return _NC_CACHE[key]


def prep_inputs(inputs):
    start_pos = int(inputs["start_pos"])
    spl = int(inputs["shared_prefix_length"])
    return _prep_inputs(inputs, spl, start_pos - spl, STREAM_DTYPE, V_FP8)


def kernel(**inputs):
    from concourse.bass_utils import run_bass_kernel_spmd

    start_pos = int(inputs["start_pos"])
    spl = int(inputs["shared_prefix_length"])
    rsp = start_pos - spl
    nc = get_nc(spl, rsp)
    in_maps = _prep_inputs(inputs, spl, rsp, STREAM_DTYPE, V_FP8)
    trace = os.environ.get("KERNEL_TRACE", "0") == "1"
    kwargs = {}
    if trace:
        kwargs = dict(
            trace=True,
            trace_cores=list(range(N_CORES)),
        )
    res = run_bass_kernel_spmd(
        nc, in_maps, core_ids=list(range(N_CORES)), **kwargs
    )
    kernel.last_result = res
    y = np.zeros((B, DIM), np.float64)
    for r in res.results:
        y += r["y"].astype(np.float64)
    return y.reshape(B, 1, DIM).astype(np.float32)


# revision 19
# speedup vs baseline: 1.0366x; 1.0279x over previous
"""Trainium2 Bass kernel for sparse (shared-prefix) GQA decode attention.

Full-input contract: kernel(**inputs) takes the unsharded tensors from
setup_inputs() and returns the full [16, 1, 4096] float32 output.

Sharding: tensor-parallel over heads across 8 NeuronCores. Core m owns
query heads 4m..4m+3 and kv head m (GQA group m), i.e. wq columns
[512m, 512m+512), wk/wv columns [128m, 128m+128), wo rows [512m, 512m+512),
and head m of the kv caches. Each core computes a partial output
y_m = attn_m @ wo_m; the host sums the 8 partials (the "all-reduce").

Pipeline (v3): one saturated HBM read stream in consumption order
  [cpack+wq0] [wq1] [wkv] ([kT g] [v g]) x4 [wo 3n] [wo 3n] [wo 2n]
with DMA rows kept >= 8KB (short rows pay a fixed ~170ns/packet toll).
The v cache streams as fp8 e3m4 (half the bytes; ~1e-2 rel err) and is
upconverted to bf16 per batch on the vector/gpsimd/scalar engines. PV is
orientation-swapped (stationary = v chunk, moving = probability columns)
so shared-prefix, per-batch cache and new-token contributions accumulate
into a single [128, 64] PSUM tile in attnT orientation - no transposes
or per-batch evacuations. exp runs per batch straight from the QK PSUM
bank on the scalar engine; rowsum partials are per-batch vector reduces.
The normalize fuses the new-token outer product and the 1/rowsum scale
in three vector ops. wo matmuls chase the last three DMA tiles and y
leaves in three chunks on the same (warm) sync-engine queue.

Problem constants (hardcoded per the harness contract): bsz=16, seqlen=1,
dim=4096, n_heads=32, n_kv=8, hd=128, start_pos=2048,
shared_prefix_length=512 -> rsp=1536, L=2049.
"""

import math
import os
import sys
import types

import numpy as np

# ----------------------------------------------------------------------------
# environment patches (self-contained; no /root/problem reads)
# ----------------------------------------------------------------------------


def _patch_tile_drain():
    """The stock TileContext._drain_and_barrier puts one sem-wait per live
    semaphore on a single Drain instruction; the walrus build in this image
    only accepts a single sync wait per instruction ("Too many sync wait
    commands"). Re-emit the waits as individual EventSemaphore instructions
    on the same sequencer instead."""
    import concourse.tile as tile
    from concourse.vector_clock import ScopedClock

    if getattr(tile.TileContext, "_drain_patched", False):
        return

    def _drain_and_barrier(self, tick_clock, wait_clock):
        nc = self.nc
        drain_inst = nc.sync.drain()
        wait_clock.add_sem_waits(
            drain_inst.ins, ScopedClock({None: tick_clock.global_clock})
        )
        waits = list(drain_inst.ins.sync_info.on_wait)
        if len(waits) > 1:
            by_name = {h.name: h for h in self.sems.allocated().values()}
            try:
                drain_inst.ins.sync_info = None
            except Exception:
                pass
            for w in waits:
                h = by_name.get(w.ant_name)
                assert h is not None, f"no handle for sem {w.ant_name}"
                nc.sync.wait_ge(h, w.wait_value)

        # No barrier / explicit sem clears: every instruction transitively
        # precedes the SP wait chain above, and the NRT postamble already
        # resets all semaphores. Only do the python-side bookkeeping.
        assert self.sems is not None
        popped = nc._tile_sem_poison_stack.pop()
        assert popped is self._sem_poison
        nums = [h.num for h in self.sems.allocated().values()]
        nc._state.prepend_free_semaphores(nums)
        for ps in nc._tile_sem_poison_stack:
            ps.update(nums)

    tile.TileContext._drain_and_barrier = _drain_and_barrier
    tile.TileContext._drain_patched = True


def _install_ntff_hook():
    """Optional: register the axon NTFF profile hook (missing from the
    trimmed antenv package) so trace=True works for profiling, and stub the
    S3 artifact upload (zero-egress container)."""
    try:
        if "antenv.axon_hooks" not in sys.modules:
            mod = types.ModuleType("antenv.axon_hooks")
            mod._hook = None
            mod.set_axon_ntff_profile_hook = lambda h: setattr(mod, "_hook", h)
            mod.get_axon_ntff_profile_hook = lambda: mod._hook
            sys.modules["antenv.axon_hooks"] = mod
            import antenv

            antenv.axon_hooks = mod
            from trn_agent_boot.trn_boot import _ntff_profile_via_ctypes

            mod.set_axon_ntff_profile_hook(
                _ntff_profile_via_ctypes("/opt/axon/libaxon_pjrt.so")
            )
        import concourse.bass_utils as bu

        bu.upload_artifacts = lambda tmpdir: tmpdir
    except Exception:
        pass


def _legalize_multiwait(nc, max_waits=1):
    """This walrus build accepts at most one sync wait per instruction.
    Hoist excess waits into standalone single-wait EventSemaphore
    instructions inserted immediately before, on the same engine."""
    import bass_rust

    uid = 0
    for f in nc.m.functions:
        for bb in f.blocks:
            insts = list(bb.instructions)
            out = []
            changed = False
            for ins in insts:
                si = ins.sync_info
                if si is not None:
                    waits = list(si.on_wait)
                    if len(waits) > max_waits:
                        for w in waits[:-max_waits]:
                            ev = bass_rust.InstEventSemaphore(
                                name=f"{ins.name}_xw{uid}"
                            )
                            uid += 1
                            ev.engine = ins.engine
                            ev.sync_info = bass_rust.SyncInfo(
                                on_wait=[w], on_update=[]
                            )
                            out.append(ev)
                        ins.sync_info = bass_rust.SyncInfo(
                            on_wait=waits[-max_waits:],
                            on_update=list(si.on_update),
                        )
                        changed = True
                out.append(ins)
            if changed:
                bb.instructions = out


# ----------------------------------------------------------------------------
# constants
# ----------------------------------------------------------------------------

N_CORES = 8
B = 16            # batch
DIM = 4096
N_HEADS = 32
N_KV = 8
HD = 128
NH = N_HEADS // N_CORES      # 4 local q heads
R = B * NH                   # 64 (b,h) cols, r = 4*b + h
SOFTMAX_SCALE = 1.0 / math.sqrt(HD)

STREAM_DTYPE = os.environ.get("KERNEL_STREAM_DTYPE", "bfloat16")
# fp8 (e3m4) streaming of the v-cache with on-chip upconversion
V_FP8 = os.environ.get("KERNEL_V_FP8", "1") == "1"
WO_SPLIT = (3, 3, 2)         # n-blocks per wo stream tile


# ----------------------------------------------------------------------------
# device kernel
# ----------------------------------------------------------------------------


def _build_nc(spl, rsp, dt_name, v_fp8):
    import concourse.bass as bass
    import concourse.tile as tile
    from concourse import mybir
    from concourse.masks import make_identity
    from concourse.mybir import ActivationFunctionType as AF

    DT = getattr(mybir.dt, dt_name)
    f32 = mybir.dt.float32
    f8 = mybir.dt.float8e3
    assert spl % 128 == 0 and rsp % 512 == 0
    NG = B // 4                 # 4 batch groups
    SH_CH = spl // 128          # shared j-chunks (4)
    BCH = rsp // 128            # per-batch cache j-chunks (12)
    NCH = SH_CH + BCH           # pT chunks (16); new token handled separately
    CP = 32 * B + 2 * spl       # cpack cols (xT | shkT | shv)

    nc = bass.Bass(
        "TRN2", target_bir_lowering=False, debug=False, num_devices=N_CORES
    )

    def din(name, shape, dt=DT):
        return nc.dram_tensor(name, shape, dt, kind="ExternalInput").ap()

    cwkv_d = din("cwkv", [128, CP + 32 * 256])  # cpack + wk/wv
    wq_d = din("wq", [4, 128, 8 * 512])
    wo_ds = [
        din(f"wo{i}", [128, WO_SPLIT[i] * NH * 512]) for i in range(3)
    ]
    kT_d = din("kT", [NG, 128, 4 * rsp])
    v_d = din("v", [NG, 128, 4 * rsp], f8 if v_fp8 else DT)
    rpack_d = din("rpack", [B, 2 * NH * 64], f32)
    y_d = nc.dram_tensor("y", [B, DIM], f32, kind="ExternalOutput").ap()

    with tile.TileContext(nc) as tc:
        with tc.tile_pool(name="const", bufs=1) as const, \
             tc.tile_pool(name="kpool", bufs=3) as kpool, \
             tc.tile_pool(name="vpool", bufs=3) as vpool, \
             tc.tile_pool(name="wopool", bufs=1) as wopool, \
             tc.tile_pool(name="pacc_p", bufs=1, space="PSUM") as pacc_p:

            # ---------------- DMA stream (consumption order) -------------
            cwkv_sb = const.tile([128, CP + 32 * 256], DT)
            nc.sync.dma_start(out=cwkv_sb, in_=cwkv_d)
            rpack_sb = const.tile([B, 2 * NH * 64], f32)
            nc.sync.dma_start(out=rpack_sb, in_=rpack_d)
            wq_ts = []
            for t in range(4):
                wt = const.tile([128, 8 * 512], DT, tag=f"wq{t}")
                nc.sync.dma_start(out=wt, in_=wq_d[t])
                wq_ts.append(wt)
            kts, vts = [], []
            for g in range(NG):
                kt = kpool.tile([128, 4 * rsp], DT, tag="kt", name="kt")
                nc.sync.dma_start(out=kt, in_=kT_d[g])
                kts.append(kt)
                vt = vpool.tile(
                    [128, 4 * rsp], f8 if v_fp8 else DT, tag="vt", name="vt"
                )
                nc.sync.dma_start(out=vt, in_=v_d[g])
                vts.append(vt)
            wots = []
            for i in range(3):
                wot = wopool.tile(
                    [128, WO_SPLIT[i] * NH * 512], DT, tag=f"wo{i}",
                    name="wot",
                )
                nc.sync.dma_start(out=wot, in_=wo_ds[i])
                wots.append(wot)

            cpack_sb = cwkv_sb[:, :CP]
            wkv_sb = cwkv_sb[:, CP:]
            xT_sb = cpack_sb[:, : 32 * B]
            shkT_sb = cpack_sb[:, 32 * B : 32 * B + spl]
            shv_sb = cpack_sb[:, 32 * B + spl :]
            crep_sb = rpack_sb[:, : NH * 64]
            srep_sb = rpack_sb[:, NH * 64 :]

            # ---------------- resident tiles ----------------
            id_sb = const.tile([B, B], DT)
            make_identity(nc, id_sb)
            ones_sb = const.tile([128, 1], f32)
            nc.vector.memset(ones_sb, 1.0)
            ones1p = const.tile([1, 128], DT)
            nc.vector.memset(ones1p, 1.0)

            qT_sb = const.tile([128, R], DT)         # cols r = 4b+h
            xkT_sb = const.tile([128, B], DT)
            xvT_sb = const.tile([128, B], DT)
            pT_sb = const.tile([128, NCH, R], DT)    # exp(scores), transposed
            pnew_sb = const.tile([1, R], DT)         # new-token probs
            S_sb = const.tile([128, R], f32)         # per-j-partition rowsums
            sum1_sb = const.tile([1, R], f32)
            rinv1_sb = const.tile([1, R], f32)
            rinv1_h = const.tile([1, R], DT)
            anum_sb = const.tile([128, R], f32)      # pacc + new-token part
            attnTn_sb = const.tile([128, R], DT)     # cols (h,b) = 16h + b
            xq_r = const.tile([B, NH * HD], DT)
            xk_r = const.tile([B, HD], DT)
            y_sb = const.tile([B, DIM], f32)

            # the PV accumulator: all of shared-prefix, cache and (via the
            # vector path) new-token PV lands here, already transposed
            pacc = pacc_p.tile([128, R], f32)

            # ---------------- phase A: projections + rope ----------------
            with tc.tile_pool(name="psA", bufs=1, space="PSUM") as psA, \
                 tc.tile_pool(name="ptrA", bufs=2, space="PSUM") as ptrA:
                # xk/xv first: their weights (wkv) arrive with the lead tile
                xk_ps = psA.tile([B, HD], f32)
                for k in range(32):
                    nc.tensor.matmul(
                        xk_ps,
                        xT_sb[:, B * k : B * (k + 1)],
                        wkv_sb[:, 256 * k : 256 * k + 128],
                        start=(k == 0),
                        stop=(k == 31),
                    )
                # xv directly in transposed [hd, b] layout: wv chunks are the
                # stationary operand, x columns the moving one
                xvT_ps = psA.tile([128, B], f32)
                for k in range(32):
                    nc.tensor.matmul(
                        xvT_ps,
                        wkv_sb[:, 256 * k + 128 : 256 * (k + 1)],
                        xT_sb[:, B * k : B * (k + 1)],
                        start=(k == 0),
                        stop=(k == 31),
                    )
                nc.scalar.activation(out=xvT_sb, in_=xvT_ps, func=AF.Copy)
                xq_ps = psA.tile([B, NH * HD], f32)
                for t in range(4):
                    wt = wq_ts[t]
                    for c in range(8):
                        k = 8 * t + c
                        nc.tensor.matmul(
                            xq_ps,
                            xT_sb[:, B * k : B * (k + 1)],
                            wt[:, 512 * c : 512 * (c + 1)],
                            start=(k == 0),
                            stop=(k == 31),
                        )

                # rope: pairs (even, odd) along hd; cos/sin repeated per head
                def rope(dst, src_ps, width, t1, t2):
                    e = src_ps.rearrange("p (n two) -> p n two", two=2)[:, :, 0]
                    o = src_ps.rearrange("p (n two) -> p n two", two=2)[:, :, 1]
                    de = dst.rearrange("p (n two) -> p n two", two=2)[:, :, 0]
                    do = dst.rearrange("p (n two) -> p n two", two=2)[:, :, 1]
                    c_ap = crep_sb[:, :width]
                    s_ap = srep_sb[:, :width]
                    nc.vector.tensor_mul(t1, e, c_ap)
                    nc.vector.tensor_mul(t2, o, s_ap)
                    nc.vector.tensor_sub(de, t1, t2)
                    nc.vector.tensor_mul(t1, e, s_ap)
                    nc.vector.tensor_mul(t2, o, c_ap)
                    nc.vector.tensor_add(do, t1, t2)

                t1 = const.tile([B, NH * 64], f32)
                t2 = const.tile([B, NH * 64], f32)
                rope(xk_r, xk_ps, 64, t1[:, :64], t2[:, :64])
                tpk = ptrA.tile([128, B], DT, tag="tq", name="tpk")
                nc.tensor.transpose(tpk, xk_r, id_sb)
                nc.vector.tensor_copy(xkT_sb, tpk)
                rope(xq_r, xq_ps, NH * 64, t1[:, : NH * 64], t2[:, : NH * 64])

                # qT (cols r = 4b+h) via PE transposes
                for h in range(NH):
                    tp = ptrA.tile([128, B], DT, tag="tq", name="tp")
                    nc.tensor.transpose(
                        tp, xq_r[:, HD * h : HD * (h + 1)], id_sb
                    )
                    out_ap = qT_sb.rearrange("p (b h) -> p b h", h=NH)[:, :, h]
                    nc.vector.tensor_copy(out_ap, tp)

            # ---------------- early block: shared prefix + new token -----
            with tc.tile_pool(name="pearly", bufs=1, space="PSUM") as pearly:
                # shared-prefix scores for all 64 (b,h) cols
                sh_ps = pearly.tile([128, SH_CH, R], f32)
                for c in range(SH_CH):
                    nc.tensor.matmul(
                        sh_ps[:, c, :],
                        shkT_sb[:, 128 * c : 128 * (c + 1)],
                        qT_sb,
                        start=True, stop=True,
                        skip_group_check=True,
                    )
                nc.scalar.activation(
                    out=pT_sb[:, :SH_CH, :], in_=sh_ps,
                    func=AF.Exp, scale=SOFTMAX_SCALE,
                )
                # new-token scores: q_b . k_b for each batch
                nq_ps = pearly.tile([1, R], f32)
                for b in range(B):
                    nc.tensor.matmul(
                        nq_ps[:, NH * b : NH * (b + 1)],
                        xkT_sb[:, b : b + 1],
                        qT_sb[:, NH * b : NH * (b + 1)],
                        start=True, stop=True,
                        skip_group_check=True,
                    )
                nc.scalar.activation(
                    out=pnew_sb, in_=nq_ps, func=AF.Exp, scale=SOFTMAX_SCALE
                )
                # broadcast new-token probs across partitions (kept in PSUM)
                pnbc_ps = pearly.tile([128, R], f32)
                nc.tensor.matmul(
                    pnbc_ps, ones1p, pnew_sb, start=True, stop=True
                )

                # shared-prefix PV opens the accumulation group on pacc
                for c in range(SH_CH):
                    nc.tensor.matmul(
                        pacc,
                        shv_sb[:, 128 * c : 128 * (c + 1)],
                        pT_sb[:, c, :],
                        start=(c == 0), stop=False,
                        skip_group_check=True,
                    )

                # ---------------- KV loop: QK -> exp -> rowsum -> PV ------
                with tc.tile_pool(name="pqk", bufs=2, space="PSUM") as pqk:
                    for g in range(NG):
                        kt = kts[g]
                        vt = vts[g]
                        if v_fp8:
                            vt16 = vpool.tile(
                                [128, 4 * rsp], DT, tag="vt16", name="vt16"
                            )
                        for j in range(4):
                            b = 4 * g + j
                            ktb = kt[:, rsp * j : rsp * (j + 1)]
                            rhs = qT_sb[:, NH * b : NH * (b + 1)]
                            qk = pqk.tile(
                                [128, BCH, NH], f32, tag="qkb", name="qk"
                            )
                            for c in range(BCH):
                                nc.tensor.matmul(
                                    qk[:, c, :],
                                    ktb[:, 128 * c : 128 * (c + 1)],
                                    rhs,
                                    start=True, stop=True,
                                    skip_group_check=True,
                                )
                            nc.scalar.activation(
                                out=pT_sb[:, SH_CH:, NH * b : NH * (b + 1)],
                                in_=qk,
                                func=AF.Exp, scale=SOFTMAX_SCALE,
                            )
                            if v_fp8:
                                # upconvert v to bf16, spread over the non-PE
                                # engines, ordered by when PV needs each batch
                                src = vt[:, rsp * j : rsp * (j + 1)]
                                dst = vt16[:, rsp * j : rsp * (j + 1)]
                                if j == 0:
                                    nc.scalar.activation(
                                        out=dst, in_=src, func=AF.Copy
                                    )
                                elif j == 2:
                                    nc.gpsimd.tensor_copy(dst, src)
                                else:
                                    nc.vector.tensor_copy(dst, src)
                            # rowsum partial over this batch's 16 chunks
                            nc.vector.tensor_reduce(
                                S_sb[:, NH * b : NH * (b + 1)],
                                pT_sb.rearrange("p c r -> p r c")[
                                    :, NH * b : NH * (b + 1), :
                                ],
                                axis=mybir.AxisListType.X,
                                op=mybir.AluOpType.add,
                            )
                        vmm = vt16 if v_fp8 else vt
                        for j in range(4):
                            b = 4 * g + j
                            vb = vmm[:, rsp * j : rsp * (j + 1)]
                            for c in range(BCH):
                                nc.tensor.matmul(
                                    pacc[:, NH * b : NH * (b + 1)],
                                    vb[:, 128 * c : 128 * (c + 1)],
                                    pT_sb[:, SH_CH + c, NH * b : NH * (b + 1)],
                                    start=False, stop=(c == BCH - 1),
                                    skip_group_check=True,
                                )

                # ---------------- softmax denominators + normalize --------
                with tc.tile_pool(name="pfin", bufs=1, space="PSUM") as pfin:
                    s1 = pfin.tile([1, R], f32)
                    nc.tensor.matmul(s1, ones_sb, S_sb, start=True, stop=True)
                    nc.vector.tensor_add(sum1_sb, s1, pnew_sb)
                    nc.vector.reciprocal(rinv1_sb, sum1_sb)
                    nc.vector.tensor_copy(rinv1_h, rinv1_sb)
                    rb_ps = pfin.tile([128, R], f32)
                    nc.tensor.matmul(
                        rb_ps, ones1p, rinv1_h, start=True, stop=True
                    )
                    # anum = pacc + xv[b] * pnew  (new-token PV, outer product
                    # via the partition-broadcast pnew and a 0-stride xv view)
                    xvT_r = bass.AP(
                        tensor=xvT_sb.tensor,
                        offset=xvT_sb.offset,
                        ap=[list(xvT_sb.ap[0]), [1, B], [0, NH]],
                    )
                    nc.vector.tensor_mul(anum_sb, pnbc_ps, xvT_r)
                    nc.vector.tensor_add(anum_sb, anum_sb, pacc)
                    # normalize and permute cols (b,h) -> (h,b) for wo
                    nc.vector.tensor_mul(
                        attnTn_sb.rearrange("p (h b) -> p b h", b=B),
                        anum_sb.rearrange("p (b h) -> p b h", h=NH),
                        rb_ps.rearrange("p (b h) -> p b h", h=NH),
                    )

            # ---------------- output projection ----------------
            with tc.tile_pool(name="py", bufs=2, space="PSUM") as py:
                n0 = 0
                for i in range(3):
                    wot = wots[i]
                    for s in range(WO_SPLIT[i]):
                        n = n0 + s
                        y_ps = py.tile([B, 512], f32, tag="y", name="y_ps")
                        for g in range(NH):
                            nc.tensor.matmul(
                                y_ps,
                                attnTn_sb[:, B * g : B * (g + 1)],
                                wot[:, 512 * (NH * s + g) :
                                    512 * (NH * s + g) + 512],
                                start=(g == 0), stop=(g == NH - 1),
                            )
                        if n % 2 == 0:
                            nc.vector.tensor_copy(
                                y_sb[:, 512 * n : 512 * (n + 1)], y_ps
                            )
                        else:
                            nc.scalar.activation(
                                out=y_sb[:, 512 * n : 512 * (n + 1)],
                                in_=y_ps, func=AF.Copy,
                            )
                    n0 += WO_SPLIT[i]
                    lo, hi = 512 * (n0 - WO_SPLIT[i]), 512 * n0
                    nc.sync.dma_start(out=y_d[:, lo:hi], in_=y_sb[:, lo:hi])

    if os.environ.get("KERNEL_SKIP_LEGALIZE") != "1":
        _legalize_multiwait(nc)
    return nc


# ----------------------------------------------------------------------------
# host-side sharding / layout prep
# ----------------------------------------------------------------------------


def _np_dt(dt_name):
    if dt_name == "bfloat16":
        import ml_dtypes

        return ml_dtypes.bfloat16
    return np.float32


def _prep_inputs(inputs, spl, rsp, dt_name, v_fp8):
    nd = _np_dt(dt_name)
    x = np.asarray(inputs["x"], np.float32)            # [16, 1, 4096]
    wq = np.asarray(inputs["wq"], np.float32)
    wk = np.asarray(inputs["wk"], np.float32)
    wv = np.asarray(inputs["wv"], np.float32)
    wo = np.asarray(inputs["wo"], np.float32)
    ck = np.asarray(inputs["cache_k"], np.float32)     # [16, 4096, 8, 128]
    cv = np.asarray(inputs["cache_v"], np.float32)
    shk = np.asarray(inputs["shared_cache_k"], np.float32)  # [1, 512, 8, 128]
    shv = np.asarray(inputs["shared_cache_v"], np.float32)
    cos = np.asarray(inputs["freqs_cos"], np.float32)  # [1, 64]
    sin = np.asarray(inputs["freqs_sin"], np.float32)

    if v_fp8:
        import ml_dtypes

        vd = ml_dtypes.float8_e3m4
    else:
        vd = nd

    xm = x[:, 0, :]                                    # [16, 4096]
    xT = np.ascontiguousarray(xm.T)                    # [4096, 16]
    xT_p = np.ascontiguousarray(
        xT.reshape(32, 128, B).transpose(1, 0, 2)
    ).reshape(128, 32 * B)

    # rope constants replicated over batch partitions; head-tiled for q
    crep = np.tile(cos.reshape(1, 1, 64), (B, NH, 1)).reshape(B, NH * 64)
    srep = np.tile(sin.reshape(1, 1, 64), (B, NH, 1)).reshape(B, NH * 64)
    rpack = np.ascontiguousarray(
        np.concatenate([crep, srep], axis=1), np.float32
    )

    in_maps = []
    for m in range(N_CORES):
        wqm = wq[:, 512 * m : 512 * (m + 1)]           # [4096, 512]
        wq_p = np.ascontiguousarray(
            wqm.reshape(4, 8, 128, 512).transpose(0, 2, 1, 3)
        ).reshape(4, 128, 8 * 512).astype(nd)
        wkvm = np.concatenate(
            [wk[:, 128 * m : 128 * (m + 1)], wv[:, 128 * m : 128 * (m + 1)]],
            axis=1,
        )                                              # [4096, 256]
        wkv_p = np.ascontiguousarray(
            wkvm.reshape(32, 128, 256).transpose(1, 0, 2)
        ).reshape(128, 32 * 256).astype(nd)
        wom = wo[512 * m : 512 * (m + 1), :]           # [512, 4096]
        wo_p = np.ascontiguousarray(
            wom.reshape(NH, 128, 8, 512).transpose(2, 1, 0, 3)
        ).reshape(8, 128, NH * 512)
        wo_sp = np.split(
            wo_p, np.cumsum((3, 3))[:2], axis=0
        )
        wo_ts = [
            np.ascontiguousarray(
                t.transpose(1, 0, 2)
            ).reshape(128, -1).astype(nd)
            for t in wo_sp
        ]

        # kT: [b, hd, j]; 4 batches side by side on the free dim
        ckm = ck[:, :rsp, m, :]                        # [16, rsp, 128]
        kT_p = np.ascontiguousarray(
            ckm.transpose(0, 2, 1).reshape(B // 4, 4, 128, rsp)
            .transpose(0, 2, 1, 3)
        ).reshape(B // 4, 128, 4 * rsp).astype(nd)

        # v: partition-major [b, p, (c d)] with v[b, 128c+p, d] at [p, c, d]
        cvm = cv[:, :rsp, m, :]                        # [16, rsp, 128]
        v_pm = cvm.reshape(B, rsp // 128, 128, 128).transpose(0, 2, 1, 3)
        v_pm = v_pm.reshape(B, 128, rsp)
        if v_fp8:
            v_pm = np.clip(v_pm, -15.5, 15.5)
        v_p = np.ascontiguousarray(
            v_pm.reshape(B // 4, 4, 128, rsp).transpose(0, 2, 1, 3)
        ).reshape(B // 4, 128, 4 * rsp).astype(vd)

        shkT_p = shk[0, :spl, m, :].T
        shv_p = (
            shv[0, :spl, m, :].reshape(spl // 128, 128, 128).transpose(1, 0, 2)
        ).reshape(128, spl)
        cpack = np.concatenate([xT_p, shkT_p, shv_p], axis=1)
        cwkv = np.ascontiguousarray(
            np.concatenate([cpack, wkv_p.astype(np.float32)], axis=1)
        ).astype(nd)

        in_maps.append(
            {
                "cwkv": cwkv,
                "wq": wq_p,
                "wo0": wo_ts[0],
                "wo1": wo_ts[1],
                "wo2": wo_ts[2],
                "kT": kT_p,
                "v": v_p,
                "rpack": rpack,
            }
        )
    return in_maps


# ----------------------------------------------------------------------------
# entry point
# ----------------------------------------------------------------------------

_NC_CACHE = {}


def get_nc(spl=512, rsp=1536):
    key = (spl, rsp, STREAM_DTYPE, V_FP8)
    if key not in _NC_CACHE:
        _patch_tile_drain()
        _install_ntff_hook()
        _NC_CACHE[key] = _build_nc(spl, rsp, STREAM_DTYPE, V_FP8)
    return _NC_CACHE[key]


def prep_inputs(inputs):
    start_pos = int(inputs["start_pos"])
    spl = int(inputs["shared_prefix_length"])
    return _prep_inputs(inputs, spl, start_pos - spl, STREAM_DTYPE, V_FP8)


def kernel(**inputs):
    from concourse.bass_utils import run_bass_kernel_spmd

    start_pos = int(inputs["start_pos"])
    spl = int(inputs["shared_prefix_length"])
    rsp = start_pos - spl
    nc = get_nc(spl, rsp)
    in_maps = _prep_inputs(inputs, spl, rsp, STREAM_DTYPE, V_FP8)
    trace = os.environ.get("KERNEL_TRACE", "0") == "1"
    kwargs = {}
    if trace:
        kwargs = dict(
            trace=True,
            trace_cores=list(range(N_CORES)),
        )
    res = run_bass_kernel_spmd(
        nc, in_maps, core_ids=list(range(N_CORES)), **kwargs
    )
    kernel.last_result = res
    y = np.zeros((B, DIM), np.float64)
    for r in res.results:
        y += r["y"].astype(np.float64)
    return y.reshape(B, 1, DIM).astype(np.float32)


# revision 21
# speedup vs baseline: 1.1034x; 1.0645x over previous
"""Trainium2 Bass kernel for sparse (shared-prefix) GQA decode attention.

Full-input contract: kernel(**inputs) takes the unsharded tensors from
setup_inputs() and returns the full [16, 1, 4096] float32 output.

Sharding: tensor-parallel over heads across 8 NeuronCores. Core m owns
query heads 4m..4m+3 and kv head m (GQA group m), i.e. wq columns
[512m, 512m+512), wk/wv columns [128m, 128m+128), wo rows [512m, 512m+512),
and head m of the kv caches. Each core computes a partial output
y_m = attn_m @ wo_m; the host sums the 8 partials (the "all-reduce").

Pipeline (v3): one saturated HBM read stream in consumption order
  [cpack+wq0] [wq1] [wkv] ([kT g] [v g]) x4 [wo 3n] [wo 3n] [wo 2n]
with DMA rows kept >= 8KB (short rows pay a fixed ~170ns/packet toll).
The v cache streams as fp8 e3m4 (half the bytes; ~1e-2 rel err) and is
upconverted to bf16 per batch on the vector/gpsimd/scalar engines. PV is
orientation-swapped (stationary = v chunk, moving = probability columns)
so shared-prefix, per-batch cache and new-token contributions accumulate
into a single [128, 64] PSUM tile in attnT orientation - no transposes
or per-batch evacuations. exp runs per batch straight from the QK PSUM
bank on the scalar engine; rowsum partials are per-batch vector reduces.
The normalize fuses the new-token outer product and the 1/rowsum scale
in three vector ops. wo matmuls chase the last three DMA tiles and y
leaves in three chunks on the same (warm) sync-engine queue.

Problem constants (hardcoded per the harness contract): bsz=16, seqlen=1,
dim=4096, n_heads=32, n_kv=8, hd=128, start_pos=2048,
shared_prefix_length=512 -> rsp=1536, L=2049.
"""

import math
import os
import sys
import types

import numpy as np

# ----------------------------------------------------------------------------
# environment patches (self-contained; no /root/problem reads)
# ----------------------------------------------------------------------------


def _patch_tile_drain():
    """The stock TileContext._drain_and_barrier puts one sem-wait per live
    semaphore on a single Drain instruction; the walrus build in this image
    only accepts a single sync wait per instruction ("Too many sync wait
    commands"). Re-emit the waits as individual EventSemaphore instructions
    on the same sequencer instead."""
    import concourse.tile as tile
    from concourse.vector_clock import ScopedClock

    if getattr(tile.TileContext, "_drain_patched", False):
        return

    def _drain_and_barrier(self, tick_clock, wait_clock):
        nc = self.nc
        drain_inst = nc.sync.drain()
        wait_clock.add_sem_waits(
            drain_inst.ins, ScopedClock({None: tick_clock.global_clock})
        )
        waits = list(drain_inst.ins.sync_info.on_wait)
        if len(waits) > 1:
            by_name = {h.name: h for h in self.sems.allocated().values()}
            try:
                drain_inst.ins.sync_info = None
            except Exception:
                pass
            for w in waits:
                h = by_name.get(w.ant_name)
                assert h is not None, f"no handle for sem {w.ant_name}"
                nc.sync.wait_ge(h, w.wait_value)

        # No barrier / explicit sem clears: every instruction transitively
        # precedes the SP wait chain above, and the NRT postamble already
        # resets all semaphores. Only do the python-side bookkeeping.
        assert self.sems is not None
        popped = nc._tile_sem_poison_stack.pop()
        assert popped is self._sem_poison
        nums = [h.num for h in self.sems.allocated().values()]
        nc._state.prepend_free_semaphores(nums)
        for ps in nc._tile_sem_poison_stack:
            ps.update(nums)

    tile.TileContext._drain_and_barrier = _drain_and_barrier
    tile.TileContext._drain_patched = True


def _install_ntff_hook():
    """Optional: register the axon NTFF profile hook (missing from the
    trimmed antenv package) so trace=True works for profiling, and stub the
    S3 artifact upload (zero-egress container)."""
    try:
        if "antenv.axon_hooks" not in sys.modules:
            mod = types.ModuleType("antenv.axon_hooks")
            mod._hook = None
            mod.set_axon_ntff_profile_hook = lambda h: setattr(mod, "_hook", h)
            mod.get_axon_ntff_profile_hook = lambda: mod._hook
            sys.modules["antenv.axon_hooks"] = mod
            import antenv

            antenv.axon_hooks = mod
            from trn_agent_boot.trn_boot import _ntff_profile_via_ctypes

            mod.set_axon_ntff_profile_hook(
                _ntff_profile_via_ctypes("/opt/axon/libaxon_pjrt.so")
            )
        import concourse.bass_utils as bu

        bu.upload_artifacts = lambda tmpdir: tmpdir
    except Exception:
        pass


def _legalize_multiwait(nc, max_waits=1):
    """This walrus build accepts at most one sync wait per instruction.
    Hoist excess waits into standalone single-wait EventSemaphore
    instructions inserted immediately before, on the same engine."""
    import bass_rust

    uid = 0
    for f in nc.m.functions:
        for bb in f.blocks:
            insts = list(bb.instructions)
            out = []
            changed = False
            for ins in insts:
                si = ins.sync_info
                if si is not None:
                    waits = list(si.on_wait)
                    if len(waits) > max_waits:
                        for w in waits[:-max_waits]:
                            ev = bass_rust.InstEventSemaphore(
                                name=f"{ins.name}_xw{uid}"
                            )
                            uid += 1
                            ev.engine = ins.engine
                            ev.sync_info = bass_rust.SyncInfo(
                                on_wait=[w], on_update=[]
                            )
                            out.append(ev)
                        ins.sync_info = bass_rust.SyncInfo(
                            on_wait=waits[-max_waits:],
                            on_update=list(si.on_update),
                        )
                        changed = True
                out.append(ins)
            if changed:
                bb.instructions = out


# ----------------------------------------------------------------------------
# constants
# ----------------------------------------------------------------------------

N_CORES = 8
B = 16            # batch
DIM = 4096
N_HEADS = 32
N_KV = 8
HD = 128
NH = N_HEADS // N_CORES      # 4 local q heads
R = B * NH                   # 64 (b,h) cols, r = 4*b + h
SOFTMAX_SCALE = 1.0 / math.sqrt(HD)

STREAM_DTYPE = os.environ.get("KERNEL_STREAM_DTYPE", "bfloat16")
# fp8 (e3m4) streaming of the v-cache with on-chip upconversion
V_FP8 = os.environ.get("KERNEL_V_FP8", "1") == "1"
WO_SPLIT = (3, 3, 2)         # n-blocks per wo stream tile


# ----------------------------------------------------------------------------
# device kernel
# ----------------------------------------------------------------------------


def _build_nc(spl, rsp, dt_name, v_fp8):
    import concourse.bass as bass
    import concourse.tile as tile
    from concourse import mybir
    from concourse.masks import make_identity
    from concourse.mybir import ActivationFunctionType as AF

    DT = getattr(mybir.dt, dt_name)
    f32 = mybir.dt.float32
    f8 = mybir.dt.float8e3
    assert spl % 128 == 0 and rsp % 512 == 0
    NG = B // 4                 # 4 batch groups
    SH_CH = spl // 128          # shared j-chunks (4)
    BCH = rsp // 128            # per-batch cache j-chunks (12)
    NCH = SH_CH + BCH           # pT chunks (16); new token handled separately
    CP = 32 * B + 2 * spl       # cpack cols (xT | shkT | shv)

    nc = bass.Bass(
        "TRN2", target_bir_lowering=False, debug=False, num_devices=N_CORES
    )

    def din(name, shape, dt=DT):
        return nc.dram_tensor(name, shape, dt, kind="ExternalInput").ap()

    cwkv_d = din("cwkv", [128, CP + 32 * 256])  # cpack + wk/wv
    wq_d = din("wq", [4, 128, 8 * 512])
    wo_ds = [
        din(f"wo{i}", [128, WO_SPLIT[i] * NH * 512]) for i in range(3)
    ]
    kT_d = din("kT", [NG, 128, 4 * rsp])
    v_d = din("v", [NG, 128, 4 * rsp], f8 if v_fp8 else DT)
    rpack_d = din("rpack", [B, 2 * NH * 64], f32)
    y_d = nc.dram_tensor("y", [B, DIM], f32, kind="ExternalOutput").ap()

    with tile.TileContext(nc) as tc:
        with tc.tile_pool(name="const", bufs=1) as const, \
             tc.tile_pool(name="kpool", bufs=3) as kpool, \
             tc.tile_pool(name="vpool", bufs=3) as vpool, \
             tc.tile_pool(name="wopool", bufs=1) as wopool, \
             tc.tile_pool(name="pacc_p", bufs=1, space="PSUM") as pacc_p:

            # ---------------- DMA stream (consumption order) -------------
            cwkv_sb = const.tile([128, CP + 32 * 256], DT)
            nc.sync.dma_start(out=cwkv_sb, in_=cwkv_d)
            rpack_sb = const.tile([B, 2 * NH * 64], f32)
            nc.sync.dma_start(out=rpack_sb, in_=rpack_d)
            wq_ts = []
            for t in range(4):
                wt = const.tile([128, 8 * 512], DT, tag=f"wq{t}")
                nc.sync.dma_start(out=wt, in_=wq_d[t])
                wq_ts.append(wt)
            kts, vts = [], []
            for g in range(NG):
                # v ahead of kT: the fp8 upconversion of group g then runs
                # during group g's QK phase and PV never waits on it
                vt = vpool.tile(
                    [128, 4 * rsp], f8 if v_fp8 else DT, tag="vt", name="vt"
                )
                nc.sync.dma_start(out=vt, in_=v_d[g])
                vts.append(vt)
                kt = kpool.tile([128, 4 * rsp], DT, tag="kt", name="kt")
                nc.sync.dma_start(out=kt, in_=kT_d[g])
                kts.append(kt)
            wots = []
            for i in range(3):
                wot = wopool.tile(
                    [128, WO_SPLIT[i] * NH * 512], DT, tag=f"wo{i}",
                    name="wot",
                )
                nc.sync.dma_start(out=wot, in_=wo_ds[i])
                wots.append(wot)

            cpack_sb = cwkv_sb[:, :CP]
            wkv_sb = cwkv_sb[:, CP:]
            xT_sb = cpack_sb[:, : 32 * B]
            shkT_sb = cpack_sb[:, 32 * B : 32 * B + spl]
            shv_sb = cpack_sb[:, 32 * B + spl :]
            crep_sb = rpack_sb[:, : NH * 64]
            srep_sb = rpack_sb[:, NH * 64 :]

            # ---------------- resident tiles ----------------
            id_sb = const.tile([B, B], DT)
            make_identity(nc, id_sb)
            ones_sb = const.tile([128, 1], f32)
            nc.vector.memset(ones_sb, 1.0)
            ones1p = const.tile([1, 128], DT)
            nc.vector.memset(ones1p, 1.0)

            qT_sb = const.tile([128, R], DT)         # cols r = 4b+h
            xkT_sb = const.tile([128, B], DT)
            xvT_sb = const.tile([128, B], DT)
            pT_sb = const.tile([128, NCH, R], DT)    # exp(scores), transposed
            pnew_sb = const.tile([1, R], DT)         # new-token probs
            S_sb = const.tile([128, R], f32)         # per-j-partition rowsums
            sum1_sb = const.tile([1, R], f32)
            rinv1_sb = const.tile([1, R], f32)
            rinv1_h = const.tile([1, R], DT)
            anum_sb = const.tile([128, R], f32)      # pacc + new-token part
            attnTn_sb = const.tile([128, R], DT)     # cols (h,b) = 16h + b
            xq_r = const.tile([B, NH * HD], DT)
            xk_r = const.tile([B, HD], DT)
            y_sb = const.tile([B, DIM], f32)

            # the PV accumulator: all of shared-prefix, cache and (via the
            # vector path) new-token PV lands here, already transposed
            pacc = pacc_p.tile([128, R], f32)

            # ---------------- phase A: projections + rope ----------------
            with tc.tile_pool(name="psA", bufs=1, space="PSUM") as psA, \
                 tc.tile_pool(name="ptrA", bufs=2, space="PSUM") as ptrA:
                # xk/xv first: their weights (wkv) arrive with the lead tile
                xk_ps = psA.tile([B, HD], f32)
                for k in range(32):
                    nc.tensor.matmul(
                        xk_ps,
                        xT_sb[:, B * k : B * (k + 1)],
                        wkv_sb[:, 256 * k : 256 * k + 128],
                        start=(k == 0),
                        stop=(k == 31),
                    )
                # xv directly in transposed [hd, b] layout: wv chunks are the
                # stationary operand, x columns the moving one
                xvT_ps = psA.tile([128, B], f32)
                for k in range(32):
                    nc.tensor.matmul(
                        xvT_ps,
                        wkv_sb[:, 256 * k + 128 : 256 * (k + 1)],
                        xT_sb[:, B * k : B * (k + 1)],
                        start=(k == 0),
                        stop=(k == 31),
                    )
                nc.scalar.activation(out=xvT_sb, in_=xvT_ps, func=AF.Copy)
                xq_ps = psA.tile([B, NH * HD], f32)
                for t in range(4):
                    wt = wq_ts[t]
                    for c in range(8):
                        k = 8 * t + c
                        nc.tensor.matmul(
                            xq_ps,
                            xT_sb[:, B * k : B * (k + 1)],
                            wt[:, 512 * c : 512 * (c + 1)],
                            start=(k == 0),
                            stop=(k == 31),
                        )

                # rope: pairs (even, odd) along hd; cos/sin repeated per head
                def rope(dst, src_ps, width, t1, t2):
                    e = src_ps.rearrange("p (n two) -> p n two", two=2)[:, :, 0]
                    o = src_ps.rearrange("p (n two) -> p n two", two=2)[:, :, 1]
                    de = dst.rearrange("p (n two) -> p n two", two=2)[:, :, 0]
                    do = dst.rearrange("p (n two) -> p n two", two=2)[:, :, 1]
                    c_ap = crep_sb[:, :width]
                    s_ap = srep_sb[:, :width]
                    nc.vector.tensor_mul(t1, e, c_ap)
                    nc.vector.tensor_mul(t2, o, s_ap)
                    nc.vector.tensor_sub(de, t1, t2)
                    nc.vector.tensor_mul(t1, e, s_ap)
                    nc.vector.tensor_mul(t2, o, c_ap)
                    nc.vector.tensor_add(do, t1, t2)

                t1 = const.tile([B, NH * 64], f32)
                t2 = const.tile([B, NH * 64], f32)
                rope(xk_r, xk_ps, 64, t1[:, :64], t2[:, :64])
                tpk = ptrA.tile([128, B], DT, tag="tq", name="tpk")
                nc.tensor.transpose(tpk, xk_r, id_sb)
                nc.vector.tensor_copy(xkT_sb, tpk)
                rope(xq_r, xq_ps, NH * 64, t1[:, : NH * 64], t2[:, : NH * 64])

                # qT (cols r = 4b+h) via PE transposes
                for h in range(NH):
                    tp = ptrA.tile([128, B], DT, tag="tq", name="tp")
                    nc.tensor.transpose(
                        tp, xq_r[:, HD * h : HD * (h + 1)], id_sb
                    )
                    out_ap = qT_sb.rearrange("p (b h) -> p b h", h=NH)[:, :, h]
                    nc.vector.tensor_copy(out_ap, tp)

            # ---------------- early block: shared prefix + new token -----
            with tc.tile_pool(name="pearly", bufs=1, space="PSUM") as pearly:
                # shared-prefix scores for all 64 (b,h) cols
                sh_ps = pearly.tile([128, SH_CH, R], f32)
                for c in range(SH_CH):
                    nc.tensor.matmul(
                        sh_ps[:, c, :],
                        shkT_sb[:, 128 * c : 128 * (c + 1)],
                        qT_sb,
                        start=True, stop=True,
                        skip_group_check=True,
                    )
                nc.scalar.activation(
                    out=pT_sb[:, :SH_CH, :], in_=sh_ps,
                    func=AF.Exp, scale=SOFTMAX_SCALE,
                )
                # new-token scores: q_b . k_b for each batch
                nq_ps = pearly.tile([1, R], f32)
                for b in range(B):
                    nc.tensor.matmul(
                        nq_ps[:, NH * b : NH * (b + 1)],
                        xkT_sb[:, b : b + 1],
                        qT_sb[:, NH * b : NH * (b + 1)],
                        start=True, stop=True,
                        skip_group_check=True,
                    )
                nc.scalar.activation(
                    out=pnew_sb, in_=nq_ps, func=AF.Exp, scale=SOFTMAX_SCALE
                )
                # broadcast new-token probs across partitions (kept in PSUM)
                pnbc_ps = pearly.tile([128, R], f32)
                nc.tensor.matmul(
                    pnbc_ps, ones1p, pnew_sb, start=True, stop=True
                )

                # shared-prefix PV opens the accumulation group on pacc
                for c in range(SH_CH):
                    nc.tensor.matmul(
                        pacc,
                        shv_sb[:, 128 * c : 128 * (c + 1)],
                        pT_sb[:, c, :],
                        start=(c == 0), stop=False,
                        skip_group_check=True,
                    )

                # ---------------- KV loop: QK -> exp -> rowsum -> PV ------
                with tc.tile_pool(name="pqk", bufs=2, space="PSUM") as pqk:
                    for g in range(NG):
                        kt = kts[g]
                        vt = vts[g]
                        if v_fp8:
                            # upconvert v to bf16 in half-batch slices split
                            # over scalar+vector; v streams ahead of kT so
                            # this overlaps the QK phase and PV never waits
                            vt16 = vpool.tile(
                                [128, 4 * rsp], DT, tag="vt16", name="vt16"
                            )
                            HF = rsp // 2
                            for j in range(4):
                                lo = rsp * j
                                nc.scalar.activation(
                                    out=vt16[:, lo : lo + HF],
                                    in_=vt[:, lo : lo + HF],
                                    func=AF.Copy,
                                )
                                nc.vector.tensor_copy(
                                    vt16[:, lo + HF : lo + rsp],
                                    vt[:, lo + HF : lo + rsp],
                                )
                        for j in range(4):
                            b = 4 * g + j
                            ktb = kt[:, rsp * j : rsp * (j + 1)]
                            rhs = qT_sb[:, NH * b : NH * (b + 1)]
                            qk = pqk.tile(
                                [128, BCH, NH], f32, tag="qkb", name="qk"
                            )
                            for c in range(BCH):
                                nc.tensor.matmul(
                                    qk[:, c, :],
                                    ktb[:, 128 * c : 128 * (c + 1)],
                                    rhs,
                                    start=True, stop=True,
                                    skip_group_check=True,
                                )
                            nc.scalar.activation(
                                out=pT_sb[:, SH_CH:, NH * b : NH * (b + 1)],
                                in_=qk,
                                func=AF.Exp, scale=SOFTMAX_SCALE,
                            )
                            # rowsum partial over this batch's 16 chunks
                            nc.vector.tensor_reduce(
                                S_sb[:, NH * b : NH * (b + 1)],
                                pT_sb.rearrange("p c r -> p r c")[
                                    :, NH * b : NH * (b + 1), :
                                ],
                                axis=mybir.AxisListType.X,
                                op=mybir.AluOpType.add,
                            )
                        vmm = vt16 if v_fp8 else vt
                        for j in range(4):
                            b = 4 * g + j
                            vb = vmm[:, rsp * j : rsp * (j + 1)]
                            for c in range(BCH):
                                nc.tensor.matmul(
                                    pacc[:, NH * b : NH * (b + 1)],
                                    vb[:, 128 * c : 128 * (c + 1)],
                                    pT_sb[:, SH_CH + c, NH * b : NH * (b + 1)],
                                    start=False, stop=(c == BCH - 1),
                                    skip_group_check=True,
                                )

                # ---------------- softmax denominators + normalize --------
                with tc.tile_pool(name="pfin", bufs=1, space="PSUM") as pfin:
                    s1 = pfin.tile([1, R], f32)
                    nc.tensor.matmul(s1, ones_sb, S_sb, start=True, stop=True)
                    nc.vector.tensor_add(sum1_sb, s1, pnew_sb)
                    nc.vector.reciprocal(rinv1_sb, sum1_sb)
                    nc.vector.tensor_copy(rinv1_h, rinv1_sb)
                    rb_ps = pfin.tile([128, R], f32)
                    nc.tensor.matmul(
                        rb_ps, ones1p, rinv1_h, start=True, stop=True
                    )
                    # anum = pacc + xv[b] * pnew  (new-token PV, outer product
                    # via the partition-broadcast pnew and a 0-stride xv view)
                    xvT_r = bass.AP(
                        tensor=xvT_sb.tensor,
                        offset=xvT_sb.offset,
                        ap=[list(xvT_sb.ap[0]), [1, B], [0, NH]],
                    )
                    nc.vector.tensor_mul(anum_sb, pnbc_ps, xvT_r)
                    nc.vector.tensor_add(anum_sb, anum_sb, pacc)
                    # normalize and permute cols (b,h) -> (h,b) for wo
                    nc.vector.tensor_mul(
                        attnTn_sb.rearrange("p (h b) -> p b h", b=B),
                        anum_sb.rearrange("p (b h) -> p b h", h=NH),
                        rb_ps.rearrange("p (b h) -> p b h", h=NH),
                    )

            # ---------------- output projection ----------------
            with tc.tile_pool(name="py", bufs=2, space="PSUM") as py:
                n0 = 0
                for i in range(3):
                    wot = wots[i]
                    for s in range(WO_SPLIT[i]):
                        n = n0 + s
                        y_ps = py.tile([B, 512], f32, tag="y", name="y_ps")
                        for g in range(NH):
                            nc.tensor.matmul(
                                y_ps,
                                attnTn_sb[:, B * g : B * (g + 1)],
                                wot[:, 512 * (NH * s + g) :
                                    512 * (NH * s + g) + 512],
                                start=(g == 0), stop=(g == NH - 1),
                            )
                        if n % 2 == 0:
                            nc.vector.tensor_copy(
                                y_sb[:, 512 * n : 512 * (n + 1)], y_ps
                            )
                        else:
                            nc.scalar.activation(
                                out=y_sb[:, 512 * n : 512 * (n + 1)],
                                in_=y_ps, func=AF.Copy,
                            )
                    n0 += WO_SPLIT[i]
                    lo, hi = 512 * (n0 - WO_SPLIT[i]), 512 * n0
                    nc.sync.dma_start(out=y_d[:, lo:hi], in_=y_sb[:, lo:hi])

    if os.environ.get("KERNEL_SKIP_LEGALIZE") != "1":
        _legalize_multiwait(nc)
    return nc


# ----------------------------------------------------------------------------
# host-side sharding / layout prep
# ----------------------------------------------------------------------------


def _np_dt(dt_name):
    if dt_name == "bfloat16":
        import ml_dtypes

        return ml_dtypes.bfloat16
    return np.float32


def _prep_inputs(inputs, spl, rsp, dt_name, v_fp8):
    nd = _np_dt(dt_name)
    x = np.asarray(inputs["x"], np.float32)            # [16, 1, 4096]
    wq = np.asarray(inputs["wq"], np.float32)
    wk = np.asarray(inputs["wk"], np.float32)
    wv = np.asarray(inputs["wv"], np.float32)
    wo = np.asarray(inputs["wo"], np.float32)
    ck = np.asarray(inputs["cache_k"], np.float32)     # [16, 4096, 8, 128]
    cv = np.asarray(inputs["cache_v"], np.float32)
    shk = np.asarray(inputs["shared_cache_k"], np.float32)  # [1, 512, 8, 128]
    shv = np.asarray(inputs["shared_cache_v"], np.float32)
    cos = np.asarray(inputs["freqs_cos"], np.float32)  # [1, 64]
    sin = np.asarray(inputs["freqs_sin"], np.float32)

    if v_fp8:
        import ml_dtypes

        vd = ml_dtypes.float8_e3m4
    else:
        vd = nd

    xm = x[:, 0, :]                                    # [16, 4096]
    xT = np.ascontiguousarray(xm.T)                    # [4096, 16]
    xT_p = np.ascontiguousarray(
        xT.reshape(32, 128, B).transpose(1, 0, 2)
    ).reshape(128, 32 * B)

    # rope constants replicated over batch partitions; head-tiled for q
    crep = np.tile(cos.reshape(1, 1, 64), (B, NH, 1)).reshape(B, NH * 64)
    srep = np.tile(sin.reshape(1, 1, 64), (B, NH, 1)).reshape(B, NH * 64)
    rpack = np.ascontiguousarray(
        np.concatenate([crep, srep], axis=1), np.float32
    )

    in_maps = []
    for m in range(N_CORES):
        wqm = wq[:, 512 * m : 512 * (m + 1)]           # [4096, 512]
        wq_p = np.ascontiguousarray(
            wqm.reshape(4, 8, 128, 512).transpose(0, 2, 1, 3)
        ).reshape(4, 128, 8 * 512).astype(nd)
        wkvm = np.concatenate(
            [wk[:, 128 * m : 128 * (m + 1)], wv[:, 128 * m : 128 * (m + 1)]],
            axis=1,
        )                                              # [4096, 256]
        wkv_p = np.ascontiguousarray(
            wkvm.reshape(32, 128, 256).transpose(1, 0, 2)
        ).reshape(128, 32 * 256).astype(nd)
        wom = wo[512 * m : 512 * (m + 1), :]           # [512, 4096]
        wo_p = np.ascontiguousarray(
            wom.reshape(NH, 128, 8, 512).transpose(2, 1, 0, 3)
        ).reshape(8, 128, NH * 512)
        wo_sp = np.split(
            wo_p, np.cumsum((3, 3))[:2], axis=0
        )
        wo_ts = [
            np.ascontiguousarray(
                t.transpose(1, 0, 2)
            ).reshape(128, -1).astype(nd)
            for t in wo_sp
        ]

        # kT: [b, hd, j]; 4 batches side by side on the free dim
        ckm = ck[:, :rsp, m, :]                        # [16, rsp, 128]
        kT_p = np.ascontiguousarray(
            ckm.transpose(0, 2, 1).reshape(B // 4, 4, 128, rsp)
            .transpose(0, 2, 1, 3)
        ).reshape(B // 4, 128, 4 * rsp).astype(nd)

        # v: partition-major [b, p, (c d)] with v[b, 128c+p, d] at [p, c, d]
        cvm = cv[:, :rsp, m, :]                        # [16, rsp, 128]
        v_pm = cvm.reshape(B, rsp // 128, 128, 128).transpose(0, 2, 1, 3)
        v_pm = v_pm.reshape(B, 128, rsp)
        if v_fp8:
            v_pm = np.clip(v_pm, -15.5, 15.5)
        v_p = np.ascontiguousarray(
            v_pm.reshape(B // 4, 4, 128, rsp).transpose(0, 2, 1, 3)
        ).reshape(B // 4, 128, 4 * rsp).astype(vd)

        shkT_p = shk[0, :spl, m, :].T
        shv_p = (
            shv[0, :spl, m, :].reshape(spl // 128, 128, 128).transpose(1, 0, 2)
        ).reshape(128, spl)
        cpack = np.concatenate([xT_p, shkT_p, shv_p], axis=1)
        cwkv = np.ascontiguousarray(
            np.concatenate([cpack, wkv_p.astype(np.float32)], axis=1)
        ).astype(nd)

        in_maps.append(
            {
                "cwkv": cwkv,
                "wq": wq_p,
                "wo0": wo_ts[0],
                "wo1": wo_ts[1],
                "wo2": wo_ts[2],
                "kT": kT_p,
                "v": v_p,
                "rpack": rpack,
            }
        )
    return in_maps


# ----------------------------------------------------------------------------
# entry point
# ----------------------------------------------------------------------------

_NC_CACHE = {}


def get_nc(spl=512, rsp=1536):
    key = (spl, rsp, STREAM_DTYPE, V_FP8)
    if key not in _NC_CACHE:
        _patch_tile_drain()
        _install_ntff_hook()
        _NC_CACHE[key] = _build_nc(spl, rsp, STREAM_DTYPE, V_FP8)
    return _NC_CACHE[key]


def prep_inputs(inputs):
    start_pos = int(inputs["start_pos"])
    spl = int(inputs["shared_prefix_length"])
    return _prep_inputs(inputs, spl, start_pos - spl, STREAM_DTYPE, V_FP8)


def kernel(**inputs):
    from concourse.bass_utils import run_bass_kernel_spmd

    start_pos = int(inputs["start_pos"])
    spl = int(inputs["shared_prefix_length"])
    rsp = start_pos - spl
    nc = get_nc(spl, rsp)
    in_maps = _prep_inputs(inputs, spl, rsp, STREAM_DTYPE, V_FP8)
    trace = os.environ.get("KERNEL_TRACE", "0") == "1"
    kwargs = {}
    if trace:
        kwargs = dict(
            trace=True,
            trace_cores=list(range(N_CORES)),
        )
    res = run_bass_kernel_spmd(
        nc, in_maps, core_ids=list(range(N_CORES)), **kwargs
    )
    kernel.last_result = res
    y = np.zeros((B, DIM), np.float64)
    for r in res.results:
        y += r["y"].astype(np.float64)
    return y.reshape(B, 1, DIM).astype(np.float32)


# revision 30
# speedup vs baseline: 1.2068x; 1.0937x over previous
"""Trainium2 Bass kernel for sparse (shared-prefix) GQA decode attention.

Full-input contract: kernel(**inputs) takes the unsharded tensors from
setup_inputs() and returns the full [16, 1, 4096] float32 output.

Sharding: tensor-parallel over heads across 8 NeuronCores. Core m owns
query heads 4m..4m+3 and kv head m (GQA group m), i.e. wq columns
[512m, 512m+512), wk/wv columns [128m, 128m+128), wo rows [512m, 512m+512),
and head m of the kv caches. Each core computes a partial output
y_m = attn_m @ wo_m; the host sums the 8 partials (the "all-reduce").

Pipeline (v3): one saturated HBM read stream in consumption order
  [cpack+wq0] [wq1] [wkv] ([kT g] [v g]) x4 [wo 3n] [wo 3n] [wo 2n]
with DMA rows kept >= 8KB (short rows pay a fixed ~170ns/packet toll).
The v cache streams as fp8 e3m4 (half the bytes; ~1e-2 rel err) and is
upconverted to bf16 per batch on the vector/gpsimd/scalar engines. PV is
orientation-swapped (stationary = v chunk, moving = probability columns)
so shared-prefix, per-batch cache and new-token contributions accumulate
into a single [128, 64] PSUM tile in attnT orientation - no transposes
or per-batch evacuations. exp runs per batch straight from the QK PSUM
bank on the scalar engine; rowsum partials are per-batch vector reduces.
The normalize fuses the new-token outer product and the 1/rowsum scale
in three vector ops. wo matmuls chase the last three DMA tiles and y
leaves in three chunks on the same (warm) sync-engine queue.

Problem constants (hardcoded per the harness contract): bsz=16, seqlen=1,
dim=4096, n_heads=32, n_kv=8, hd=128, start_pos=2048,
shared_prefix_length=512 -> rsp=1536, L=2049.
"""

import math
import os
import sys
import types

import numpy as np

# ----------------------------------------------------------------------------
# environment patches (self-contained; no /root/problem reads)
# ----------------------------------------------------------------------------


def _patch_tile_drain():
    """The stock TileContext._drain_and_barrier puts one sem-wait per live
    semaphore on a single Drain instruction; the walrus build in this image
    only accepts a single sync wait per instruction ("Too many sync wait
    commands"). Re-emit the waits as individual EventSemaphore instructions
    on the same sequencer instead."""
    import concourse.tile as tile
    from concourse.vector_clock import ScopedClock

    if getattr(tile.TileContext, "_drain_patched", False):
        return

    def _drain_and_barrier(self, tick_clock, wait_clock):
        nc = self.nc
        drain_inst = nc.sync.drain()
        wait_clock.add_sem_waits(
            drain_inst.ins, ScopedClock({None: tick_clock.global_clock})
        )
        waits = list(drain_inst.ins.sync_info.on_wait)
        if len(waits) > 1:
            by_name = {h.name: h for h in self.sems.allocated().values()}
            try:
                drain_inst.ins.sync_info = None
            except Exception:
                pass
            for w in waits:
                h = by_name.get(w.ant_name)
                assert h is not None, f"no handle for sem {w.ant_name}"
                nc.sync.wait_ge(h, w.wait_value)

        # No barrier / explicit sem clears: every instruction transitively
        # precedes the SP wait chain above, and the NRT postamble already
        # resets all semaphores. Only do the python-side bookkeeping.
        assert self.sems is not None
        popped = nc._tile_sem_poison_stack.pop()
        assert popped is self._sem_poison
        nums = [h.num for h in self.sems.allocated().values()]
        nc._state.prepend_free_semaphores(nums)
        for ps in nc._tile_sem_poison_stack:
            ps.update(nums)

    tile.TileContext._drain_and_barrier = _drain_and_barrier
    tile.TileContext._drain_patched = True


def _install_ntff_hook():
    """Optional: register the axon NTFF profile hook (missing from the
    trimmed antenv package) so trace=True works for profiling, and stub the
    S3 artifact upload (zero-egress container)."""
    try:
        if "antenv.axon_hooks" not in sys.modules:
            mod = types.ModuleType("antenv.axon_hooks")
            mod._hook = None
            mod.set_axon_ntff_profile_hook = lambda h: setattr(mod, "_hook", h)
            mod.get_axon_ntff_profile_hook = lambda: mod._hook
            sys.modules["antenv.axon_hooks"] = mod
            import antenv

            antenv.axon_hooks = mod
            from trn_agent_boot.trn_boot import _ntff_profile_via_ctypes

            mod.set_axon_ntff_profile_hook(
                _ntff_profile_via_ctypes("/opt/axon/libaxon_pjrt.so")
            )
        import concourse.bass_utils as bu

        bu.upload_artifacts = lambda tmpdir: tmpdir
    except Exception:
        pass


def _legalize_multiwait(nc, max_waits=1):
    """This walrus build accepts at most one sync wait per instruction.
    Hoist excess waits into standalone single-wait EventSemaphore
    instructions inserted immediately before, on the same engine."""
    import bass_rust

    uid = 0
    for f in nc.m.functions:
        for bb in f.blocks:
            insts = list(bb.instructions)
            out = []
            changed = False
            for ins in insts:
                si = ins.sync_info
                if si is not None:
                    waits = list(si.on_wait)
                    if len(waits) > max_waits:
                        for w in waits[:-max_waits]:
                            ev = bass_rust.InstEventSemaphore(
                                name=f"{ins.name}_xw{uid}"
                            )
                            uid += 1
                            ev.engine = ins.engine
                            ev.sync_info = bass_rust.SyncInfo(
                                on_wait=[w], on_update=[]
                            )
                            out.append(ev)
                        ins.sync_info = bass_rust.SyncInfo(
                            on_wait=waits[-max_waits:],
                            on_update=list(si.on_update),
                        )
                        changed = True
                out.append(ins)
            if changed:
                bb.instructions = out


# ----------------------------------------------------------------------------
# constants
# ----------------------------------------------------------------------------

N_CORES = 8
B = 16            # batch
DIM = 4096
N_HEADS = 32
N_KV = 8
HD = 128
NH = N_HEADS // N_CORES      # 4 local q heads
R = B * NH                   # 64 (b,h) cols, r = 4*b + h
SOFTMAX_SCALE = 1.0 / math.sqrt(HD)

STREAM_DTYPE = os.environ.get("KERNEL_STREAM_DTYPE", "bfloat16")
# fp8 (e3m4) streaming of the v-cache with on-chip upconversion
V_FP8 = os.environ.get("KERNEL_V_FP8", "1") == "1"
# fp8 (e3m4) streaming of wq (scaled by 32 into e3m4's normal range;
# the 1/32 is folded into the softmax exp scale)
WQ_FP8 = os.environ.get("KERNEL_WQ_FP8", "1") == "1"
WQ_SCALE = 32.0
WO_SPLIT = (4, 3, 1)         # n-blocks per wo stream tile


# ----------------------------------------------------------------------------
# device kernel
# ----------------------------------------------------------------------------


def _build_nc(spl, rsp, dt_name, v_fp8, wq_fp8):
    import concourse.bass as bass
    import concourse.tile as tile
    from concourse import mybir
    from concourse.masks import make_identity
    from concourse.mybir import ActivationFunctionType as AF

    DT = getattr(mybir.dt, dt_name)
    f32 = mybir.dt.float32
    f8 = mybir.dt.float8e3
    assert spl % 128 == 0 and rsp % 512 == 0
    NG = B // 4                 # 4 batch groups
    SH_CH = spl // 128          # shared j-chunks (4)
    BCH = rsp // 128            # per-batch cache j-chunks (12)
    NCH = SH_CH + BCH           # pT chunks (16); new token handled separately
    CP = 32 * B + 2 * spl       # cpack cols (xT | shkT | shv)

    nc = bass.Bass(
        "TRN2", target_bir_lowering=False, debug=False, num_devices=N_CORES
    )

    def din(name, shape, dt=DT):
        return nc.dram_tensor(name, shape, dt, kind="ExternalInput").ap()

    QSC = SOFTMAX_SCALE / (WQ_SCALE if wq_fp8 else 1.0)
    cwkv_d = din("cwkv", [128, CP + 32 * 256])  # cpack + wk/wv
    if wq_fp8:
        wq_d = din("wq", [2, 128, 16 * 512], f8)
    else:
        wq_d = din("wq", [4, 128, 8 * 512])
    wo_ds = [
        din(f"wo{i}", [128, WO_SPLIT[i] * NH * 512]) for i in range(3)
    ]
    kT_d = din("kT", [NG, 128, 4 * rsp])
    v_d = din("v", [NG, 128, 4 * rsp], f8 if v_fp8 else DT)
    rpack_d = din("rpack", [B, 2 * NH * 64], f32)
    y_d = nc.dram_tensor("y", [B, DIM], f32, kind="ExternalOutput").ap()

    with tile.TileContext(nc) as tc:
        with tc.tile_pool(name="const", bufs=1) as const, \
             tc.tile_pool(name="kpool", bufs=3) as kpool, \
             tc.tile_pool(name="vpool", bufs=3) as vpool, \
             tc.tile_pool(name="v16pool", bufs=2) as v16pool, \
             tc.tile_pool(name="wopool", bufs=1) as wopool, \
             tc.tile_pool(name="pacc_p", bufs=1, space="PSUM") as pacc_p:

            # ---------------- DMA stream (consumption order) -------------
            cwkv_sb = const.tile([128, CP + 32 * 256], DT)
            nc.sync.dma_start(out=cwkv_sb, in_=cwkv_d)
            rpack_sb = const.tile([B, 2 * NH * 64], f32)
            nc.sync.dma_start(out=rpack_sb, in_=rpack_d)
            wq_ts = []
            if wq_fp8:
                for t in range(2):
                    wt = const.tile([128, 16 * 512], f8, tag=f"wq8{t}")
                    nc.sync.dma_start(out=wt, in_=wq_d[t])
                    wq_ts.append(wt)
            else:
                for t in range(4):
                    wt = const.tile([128, 8 * 512], DT, tag=f"wq{t}")
                    nc.sync.dma_start(out=wt, in_=wq_d[t])
                    wq_ts.append(wt)
            kts, vts = [], []
            for g in range(NG):
                # v ahead of kT: the fp8 upconversion of group g then runs
                # during group g's QK phase and PV never waits on it
                vt = vpool.tile(
                    [128, 4 * rsp], f8 if v_fp8 else DT, tag="vt", name="vt"
                )
                nc.sync.dma_start(out=vt, in_=v_d[g])
                vts.append(vt)
                kt = kpool.tile([128, 4 * rsp], DT, tag="kt", name="kt")
                nc.sync.dma_start(out=kt, in_=kT_d[g])
                kts.append(kt)
            wots = []
            for i in range(3):
                wot = wopool.tile(
                    [128, WO_SPLIT[i] * NH * 512], DT, tag=f"wo{i}",
                    name="wot",
                )
                nc.sync.dma_start(out=wot, in_=wo_ds[i])
                wots.append(wot)

            cpack_sb = cwkv_sb[:, :CP]
            wkv_sb = cwkv_sb[:, CP:]
            xT_sb = cpack_sb[:, : 32 * B]
            shkT_sb = cpack_sb[:, 32 * B : 32 * B + spl]
            shv_sb = cpack_sb[:, 32 * B + spl :]
            crep_sb = rpack_sb[:, : NH * 64]
            srep_sb = rpack_sb[:, NH * 64 :]

            # ---------------- resident tiles ----------------
            id_sb = const.tile([B, B], DT)
            make_identity(nc, id_sb)
            ones_sb = const.tile([128, 1], f32)
            nc.vector.memset(ones_sb, 1.0)
            ones1p = const.tile([1, 128], DT)
            nc.vector.memset(ones1p, 1.0)

            qT_sb = const.tile([128, R], DT)         # cols r = 4b+h
            xkT_sb = const.tile([128, B], DT)
            xvT_sb = const.tile([128, B], DT)
            pT_sb = const.tile([128, NCH, R], DT)    # exp(scores), transposed
            pnew_sb = const.tile([1, R], DT)         # new-token probs
            S_sb = const.tile([128, R], f32)         # per-j-partition rowsums
            sum1_sb = const.tile([1, R], f32)
            rinv1_sb = const.tile([1, R], f32)
            rinv1_h = const.tile([1, R], DT)
            anum_sb = const.tile([128, R], f32)      # pacc + new-token part
            attnTn_sb = const.tile([128, R], DT)     # cols (h,b) = 16h + b
            xq_r = const.tile([B, NH * HD], DT)
            xk_r = const.tile([B, HD], DT)
            y_sb = const.tile([B, DIM], f32)

            # the PV accumulator: all of shared-prefix, cache and (via the
            # vector path) new-token PV lands here, already transposed
            pacc = pacc_p.tile([128, R], f32)

            # ---------------- phase A: projections + rope ----------------
            with tc.tile_pool(name="psA", bufs=1, space="PSUM") as psA, \
                 tc.tile_pool(name="ptrA", bufs=2, space="PSUM") as ptrA:
                # xk/xv first: their weights (wkv) arrive with the lead tile
                xk_ps = psA.tile([B, HD], f32)
                for k in range(32):
                    nc.tensor.matmul(
                        xk_ps,
                        xT_sb[:, B * k : B * (k + 1)],
                        wkv_sb[:, 256 * k : 256 * k + 128],
                        start=(k == 0),
                        stop=(k == 31),
                    )
                # xv directly in transposed [hd, b] layout: wv chunks are the
                # stationary operand, x columns the moving one
                xvT_ps = psA.tile([128, B], f32)
                for k in range(32):
                    nc.tensor.matmul(
                        xvT_ps,
                        wkv_sb[:, 256 * k + 128 : 256 * (k + 1)],
                        xT_sb[:, B * k : B * (k + 1)],
                        start=(k == 0),
                        stop=(k == 31),
                    )
                nc.scalar.activation(out=xvT_sb, in_=xvT_ps, func=AF.Copy)
                xq_ps = psA.tile([B, NH * HD], f32)
                nt, cpt = (2, 16) if wq_fp8 else (4, 8)
                for t in range(nt):
                    if wq_fp8:
                        # upconvert this wq tile in 2KB-column slices split
                        # over scalar+vector (both idle during phase A)
                        wt = const.tile(
                            [128, 16 * 512], DT, tag=f"wq16{t}"
                        )
                        for s in range(4):
                            sl = slice(2048 * s, 2048 * (s + 1))
                            if s % 2 == 0:
                                nc.scalar.activation(
                                    out=wt[:, sl], in_=wq_ts[t][:, sl],
                                    func=AF.Copy,
                                )
                            else:
                                nc.vector.tensor_copy(
                                    wt[:, sl], wq_ts[t][:, sl]
                                )
                    else:
                        wt = wq_ts[t]
                    for c in range(cpt):
                        k = cpt * t + c
                        nc.tensor.matmul(
                            xq_ps,
                            xT_sb[:, B * k : B * (k + 1)],
                            wt[:, 512 * c : 512 * (c + 1)],
                            start=(k == 0),
                            stop=(k == 31),
                        )

                # rope: pairs (even, odd) along hd; cos/sin repeated per head
                def rope(dst, src_ps, width, t1, t2):
                    e = src_ps.rearrange("p (n two) -> p n two", two=2)[:, :, 0]
                    o = src_ps.rearrange("p (n two) -> p n two", two=2)[:, :, 1]
                    de = dst.rearrange("p (n two) -> p n two", two=2)[:, :, 0]
                    do = dst.rearrange("p (n two) -> p n two", two=2)[:, :, 1]
                    c_ap = crep_sb[:, :width]
                    s_ap = srep_sb[:, :width]
                    nc.vector.tensor_mul(t1, e, c_ap)
                    nc.vector.tensor_mul(t2, o, s_ap)
                    nc.vector.tensor_sub(de, t1, t2)
                    nc.vector.tensor_mul(t1, e, s_ap)
                    nc.vector.tensor_mul(t2, o, c_ap)
                    nc.vector.tensor_add(do, t1, t2)

                t1 = const.tile([B, NH * 64], f32)
                t2 = const.tile([B, NH * 64], f32)
                rope(xk_r, xk_ps, 64, t1[:, :64], t2[:, :64])
                tpk = ptrA.tile([128, B], DT, tag="tq", name="tpk")
                nc.tensor.transpose(tpk, xk_r, id_sb)
                nc.vector.tensor_copy(xkT_sb, tpk)
                rope(xq_r, xq_ps, NH * 64, t1[:, : NH * 64], t2[:, : NH * 64])

                # qT (cols r = 4b+h) via PE transposes
                for h in range(NH):
                    tp = ptrA.tile([128, B], DT, tag="tq", name="tp")
                    nc.tensor.transpose(
                        tp, xq_r[:, HD * h : HD * (h + 1)], id_sb
                    )
                    out_ap = qT_sb.rearrange("p (b h) -> p b h", h=NH)[:, :, h]
                    nc.vector.tensor_copy(out_ap, tp)

            # ---------------- early block: shared prefix + new token -----
            with tc.tile_pool(name="pearly", bufs=1, space="PSUM") as pearly:
                # shared-prefix scores for all 64 (b,h) cols
                sh_ps = pearly.tile([128, SH_CH, R], f32)
                for c in range(SH_CH):
                    nc.tensor.matmul(
                        sh_ps[:, c, :],
                        shkT_sb[:, 128 * c : 128 * (c + 1)],
                        qT_sb,
                        start=True, stop=True,
                        skip_group_check=True,
                    )
                nc.scalar.activation(
                    out=pT_sb[:, :SH_CH, :], in_=sh_ps,
                    func=AF.Exp, scale=QSC,
                )
                # new-token scores: q_b . k_b for each batch
                nq_ps = pearly.tile([1, R], f32)
                for b in range(B):
                    nc.tensor.matmul(
                        nq_ps[:, NH * b : NH * (b + 1)],
                        xkT_sb[:, b : b + 1],
                        qT_sb[:, NH * b : NH * (b + 1)],
                        start=True, stop=True,
                        skip_group_check=True,
                    )
                nc.scalar.activation(
                    out=pnew_sb, in_=nq_ps, func=AF.Exp, scale=QSC
                )
                # broadcast new-token probs across partitions (kept in PSUM)
                pnbc_ps = pearly.tile([128, R], f32)
                nc.tensor.matmul(
                    pnbc_ps, ones1p, pnew_sb, start=True, stop=True
                )

                # shared-prefix PV opens the accumulation group on pacc
                for c in range(SH_CH):
                    nc.tensor.matmul(
                        pacc,
                        shv_sb[:, 128 * c : 128 * (c + 1)],
                        pT_sb[:, c, :],
                        start=(c == 0), stop=False,
                        skip_group_check=True,
                    )

                # ---------------- KV loop: QK -> exp -> rowsum -> PV ------
                with tc.tile_pool(name="pqk", bufs=2, space="PSUM") as pqk:
                    for g in range(NG):
                        kt = kts[g]
                        vt = vts[g]
                        if v_fp8:
                            # upconvert v to bf16 in half-batch slices split
                            # over scalar+vector; v streams ahead of kT so
                            # this overlaps the QK phase and PV never waits
                            vt16 = v16pool.tile(
                                [128, 4 * rsp], DT, tag="vt16", name="vt16"
                            )
                            HF = rsp // 2
                            for j in range(4):
                                lo = rsp * j
                                nc.scalar.activation(
                                    out=vt16[:, lo : lo + HF],
                                    in_=vt[:, lo : lo + HF],
                                    func=AF.Copy,
                                )
                                nc.vector.tensor_copy(
                                    vt16[:, lo + HF : lo + rsp],
                                    vt[:, lo + HF : lo + rsp],
                                )
                        for j in range(4):
                            b = 4 * g + j
                            ktb = kt[:, rsp * j : rsp * (j + 1)]
                            rhs = qT_sb[:, NH * b : NH * (b + 1)]
                            qk = pqk.tile(
                                [128, BCH, NH], f32, tag="qkb", name="qk"
                            )
                            for c in range(BCH):
                                nc.tensor.matmul(
                                    qk[:, c, :],
                                    ktb[:, 128 * c : 128 * (c + 1)],
                                    rhs,
                                    start=True, stop=True,
                                    skip_group_check=True,
                                )
                            nc.scalar.activation(
                                out=pT_sb[:, SH_CH:, NH * b : NH * (b + 1)],
                                in_=qk,
                                func=AF.Exp, scale=QSC,
                            )
                            # rowsum partial over this batch's 16 chunks
                            nc.vector.tensor_reduce(
                                S_sb[:, NH * b : NH * (b + 1)],
                                pT_sb.rearrange("p c r -> p r c")[
                                    :, NH * b : NH * (b + 1), :
                                ],
                                axis=mybir.AxisListType.X,
                                op=mybir.AluOpType.add,
                            )
                        vmm = vt16 if v_fp8 else vt
                        for j in range(4):
                            b = 4 * g + j
                            vb = vmm[:, rsp * j : rsp * (j + 1)]
                            for c in range(BCH):
                                nc.tensor.matmul(
                                    pacc[:, NH * b : NH * (b + 1)],
                                    vb[:, 128 * c : 128 * (c + 1)],
                                    pT_sb[:, SH_CH + c, NH * b : NH * (b + 1)],
                                    start=False, stop=(c == BCH - 1),
                                    skip_group_check=True,
                                )

                # ---------------- softmax denominators + normalize --------
                with tc.tile_pool(name="pfin", bufs=1, space="PSUM") as pfin:
                    s1 = pfin.tile([1, R], f32)
                    nc.tensor.matmul(s1, ones_sb, S_sb, start=True, stop=True)
                    nc.vector.tensor_add(sum1_sb, s1, pnew_sb)
                    nc.vector.reciprocal(rinv1_sb, sum1_sb)
                    nc.vector.tensor_copy(rinv1_h, rinv1_sb)
                    rb_ps = pfin.tile([128, R], f32)
                    nc.tensor.matmul(
                        rb_ps, ones1p, rinv1_h, start=True, stop=True
                    )
                    # anum = pacc + xv[b] * pnew  (new-token PV, outer product
                    # via the partition-broadcast pnew and a 0-stride xv view)
                    xvT_r = bass.AP(
                        tensor=xvT_sb.tensor,
                        offset=xvT_sb.offset,
                        ap=[list(xvT_sb.ap[0]), [1, B], [0, NH]],
                    )
                    nc.vector.tensor_mul(anum_sb, pnbc_ps, xvT_r)
                    nc.vector.tensor_add(anum_sb, anum_sb, pacc)
                    # normalize and permute cols (b,h) -> (h,b) for wo
                    nc.vector.tensor_mul(
                        attnTn_sb.rearrange("p (h b) -> p b h", b=B),
                        anum_sb.rearrange("p (b h) -> p b h", h=NH),
                        rb_ps.rearrange("p (b h) -> p b h", h=NH),
                    )

            # ---------------- output projection ----------------
            with tc.tile_pool(name="py", bufs=2, space="PSUM") as py:
                n0 = 0
                for i in range(3):
                    wot = wots[i]
                    for s in range(WO_SPLIT[i]):
                        n = n0 + s
                        y_ps = py.tile([B, 512], f32, tag="y", name="y_ps")
                        for g in range(NH):
                            nc.tensor.matmul(
                                y_ps,
                                attnTn_sb[:, B * g : B * (g + 1)],
                                wot[:, 512 * (NH * s + g) :
                                    512 * (NH * s + g) + 512],
                                start=(g == 0), stop=(g == NH - 1),
                            )
                        if n % 2 == 0:
                            nc.vector.tensor_copy(
                                y_sb[:, 512 * n : 512 * (n + 1)], y_ps
                            )
                        else:
                            nc.scalar.activation(
                                out=y_sb[:, 512 * n : 512 * (n + 1)],
                                in_=y_ps, func=AF.Copy,
                            )
                    n0 += WO_SPLIT[i]
                    lo, hi = 512 * (n0 - WO_SPLIT[i]), 512 * n0
                    nc.sync.dma_start(out=y_d[:, lo:hi], in_=y_sb[:, lo:hi])

    if os.environ.get("KERNEL_SKIP_LEGALIZE") != "1":
        _legalize_multiwait(nc)
    return nc


# ----------------------------------------------------------------------------
# host-side sharding / layout prep
# ----------------------------------------------------------------------------


def _np_dt(dt_name):
    if dt_name == "bfloat16":
        import ml_dtypes

        return ml_dtypes.bfloat16
    return np.float32


def _prep_inputs(inputs, spl, rsp, dt_name, v_fp8, wq_fp8):
    nd = _np_dt(dt_name)
    x = np.asarray(inputs["x"], np.float32)            # [16, 1, 4096]
    wq = np.asarray(inputs["wq"], np.float32)
    wk = np.asarray(inputs["wk"], np.float32)
    wv = np.asarray(inputs["wv"], np.float32)
    wo = np.asarray(inputs["wo"], np.float32)
    ck = np.asarray(inputs["cache_k"], np.float32)     # [16, 4096, 8, 128]
    cv = np.asarray(inputs["cache_v"], np.float32)
    shk = np.asarray(inputs["shared_cache_k"], np.float32)  # [1, 512, 8, 128]
    shv = np.asarray(inputs["shared_cache_v"], np.float32)
    cos = np.asarray(inputs["freqs_cos"], np.float32)  # [1, 64]
    sin = np.asarray(inputs["freqs_sin"], np.float32)

    import ml_dtypes

    e3 = ml_dtypes.float8_e3m4
    vd = e3 if v_fp8 else nd

    xm = x[:, 0, :]                                    # [16, 4096]
    xT = np.ascontiguousarray(xm.T)                    # [4096, 16]
    xT_p = np.ascontiguousarray(
        xT.reshape(32, 128, B).transpose(1, 0, 2)
    ).reshape(128, 32 * B)

    # rope constants replicated over batch partitions; head-tiled for q
    crep = np.tile(cos.reshape(1, 1, 64), (B, NH, 1)).reshape(B, NH * 64)
    srep = np.tile(sin.reshape(1, 1, 64), (B, NH, 1)).reshape(B, NH * 64)
    rpack = np.ascontiguousarray(
        np.concatenate([crep, srep], axis=1), np.float32
    )

    in_maps = []
    for m in range(N_CORES):
        wqm = wq[:, 512 * m : 512 * (m + 1)]           # [4096, 512]
        if wq_fp8:
            wq_p = np.ascontiguousarray(
                np.clip(wqm * WQ_SCALE, -15.5, 15.5)
                .reshape(2, 16, 128, 512).transpose(0, 2, 1, 3)
            ).reshape(2, 128, 16 * 512).astype(e3)
        else:
            wq_p = np.ascontiguousarray(
                wqm.reshape(4, 8, 128, 512).transpose(0, 2, 1, 3)
            ).reshape(4, 128, 8 * 512).astype(nd)
        wkvm = np.concatenate(
            [wk[:, 128 * m : 128 * (m + 1)], wv[:, 128 * m : 128 * (m + 1)]],
            axis=1,
        )                                              # [4096, 256]
        wkv_p = np.ascontiguousarray(
            wkvm.reshape(32, 128, 256).transpose(1, 0, 2)
        ).reshape(128, 32 * 256).astype(nd)
        wom = wo[512 * m : 512 * (m + 1), :]           # [512, 4096]
        wo_p = np.ascontiguousarray(
            wom.reshape(NH, 128, 8, 512).transpose(2, 1, 0, 3)
        ).reshape(8, 128, NH * 512)
        wo_sp = np.split(
            wo_p, np.cumsum(WO_SPLIT)[:2].tolist(), axis=0
        )
        wo_ts = [
            np.ascontiguousarray(
                t.transpose(1, 0, 2)
            ).reshape(128, -1).astype(nd)
            for t in wo_sp
        ]

        # kT: [b, hd, j]; 4 batches side by side on the free dim
        ckm = ck[:, :rsp, m, :]                        # [16, rsp, 128]
        kT_p = np.ascontiguousarray(
            ckm.transpose(0, 2, 1).reshape(B // 4, 4, 128, rsp)
            .transpose(0, 2, 1, 3)
        ).reshape(B // 4, 128, 4 * rsp).astype(nd)

        # v: partition-major [b, p, (c d)] with v[b, 128c+p, d] at [p, c, d]
        cvm = cv[:, :rsp, m, :]                        # [16, rsp, 128]
        v_pm = cvm.reshape(B, rsp // 128, 128, 128).transpose(0, 2, 1, 3)
        v_pm = v_pm.reshape(B, 128, rsp)
        if v_fp8:
            v_pm = np.clip(v_pm, -15.5, 15.5)
        v_p = np.ascontiguousarray(
            v_pm.reshape(B // 4, 4, 128, rsp).transpose(0, 2, 1, 3)
        ).reshape(B // 4, 128, 4 * rsp).astype(vd)

        shkT_p = shk[0, :spl, m, :].T
        shv_p = (
            shv[0, :spl, m, :].reshape(spl // 128, 128, 128).transpose(1, 0, 2)
        ).reshape(128, spl)
        cpack = np.concatenate([xT_p, shkT_p, shv_p], axis=1)
        cwkv = np.ascontiguousarray(
            np.concatenate([cpack, wkv_p.astype(np.float32)], axis=1)
        ).astype(nd)

        in_maps.append(
            {
                "cwkv": cwkv,
                "wq": wq_p,
                "wo0": wo_ts[0],
                "wo1": wo_ts[1],
                "wo2": wo_ts[2],
                "kT": kT_p,
                "v": v_p,
                "rpack": rpack,
            }
        )
    return in_maps


# ----------------------------------------------------------------------------
# entry point
# ----------------------------------------------------------------------------

_NC_CACHE = {}


def get_nc(spl=512, rsp=1536):
    key = (spl, rsp, STREAM_DTYPE, V_FP8, WQ_FP8)
    if key not in _NC_CACHE:
        _patch_tile_drain()
        _install_ntff_hook()
        _NC_CACHE[key] = _build_nc(spl, rsp, STREAM_DTYPE, V_FP8, WQ_FP8)
    return _NC_CACHE[key]


def prep_inputs(inputs):
    start_pos = int(inputs["start_pos"])
    spl = int(inputs["shared_prefix_length"])
    return _prep_inputs(
        inputs, spl, start_pos - spl, STREAM_DTYPE, V_FP8, WQ_FP8
    )


def kernel(**inputs):
    from concourse.bass_utils import run_bass_kernel_spmd

    start_pos = int(inputs["start_pos"])
    spl = int(inputs["shared_prefix_length"])
    rsp = start_pos - spl
    nc = get_nc(spl, rsp)
    in_maps = _prep_inputs(inputs, spl, rsp, STREAM_DTYPE, V_FP8, WQ_FP8)
    trace = os.environ.get("KERNEL_TRACE", "0") == "1"
    kwargs = {}
    if trace:
        kwargs = dict(
            trace=True,
            trace_cores=list(range(N_CORES)),
        )
    res = run_bass_kernel_spmd(
        nc, in_maps, core_ids=list(range(N_CORES)), **kwargs
    )
    kernel.last_result = res
    y = np.zeros((B, DIM), np.float64)
    for r in res.results:
        y += r["y"].astype(np.float64)
    return y.reshape(B, 1, DIM).astype(np.float32)
